# revision 12
# baseline (speedup 1.0000x reference)
"""AttentiveChildSumTreeLSTMCell on 8 Trainium2 NeuronCores.

Tensor-parallel: column-parallel f/attention/iou linears (hidden dim sharded
8 ways), row-parallel merge linear.  Collectives: two zero-dependency warmup
AllGathers (absorb communicator init + first-collective software setup),
AllGather of partial attention logits, AllReduce of merge-linear partials +
f LayerNorm stats, AllGather of iou/forget*cell chunks.  Matmul operands
are bf16; accumulation and all norm/gate math stays fp32.

All activations use a single ACT table set (ln+exp, loaded once): sigmoid
and tanh are computed via exp + DVE fast-reciprocal, LayerNorm rstd via
exp(-0.5*ln(var+eps)).  When all LayerNorm gains are 1 and biases 0 (the
common case, verified at runtime), the (x-mean)*rstd normalization is
folded into the exp activation's per-partition scale/bias operands.  The
gpsimd queue carries only collective triggers; the final gate math runs in
a [8, 256] chunk layout read straight from the AllGather result.
"""

import sys

for _p in ("/opt/trn_rl_repo",):
    if _p not in sys.path:
        sys.path.insert(0, _p)

import ml_dtypes
import numpy as np

import concourse.bacc as bacc
import concourse.mybir as mybir
import concourse.tile as tile
from concourse.bass_utils import run_bass_kernel_spmd
from concourse.tile_rust import add_dep_helper

F32 = mybir.dt.float32
BF16 = mybir.dt.bfloat16
AF = mybir.ActivationFunctionType
ALU = mybir.AluOpType
NPBF = ml_dtypes.bfloat16

H = 2048
N = 32
NC = 8
S = H // NC           # 256: per-core chunk of every sharded dim
T = H // 128          # 16 tiles of 128 along a 2048 dim
KT = 32               # K-tiles along the 4096 contraction dims
EPS = 1e-5
INV_H = 1.0 / H

# index of the ln+exp activation-function set in act_info.json
LN_EXP_SET = 6

_CACHE = {}


def _build(trivial_ln):
    nc = bacc.Bacc(None, target_bir_lowering=False, debug=False, num_devices=NC)

    def din(name, shape, dt=F32):
        return nc.dram_tensor(name, list(shape), dt, kind="ExternalInput")

    # ---- per-core DRAM inputs (SPMD: same shapes on every core) ----
    hT = din("hT", (128, T * N), BF16)
    xT32 = din("xT32", (128, T * N), BF16)
    eT32 = din("eT32", (128, T * N), BF16)
    x1 = din("x1", (128, T), BF16)
    hTc = din("hTc", (128, 2 * N), BF16)
    cells_chunk = din("cells_chunk", (N, S))
    gf_rep = din("gf_rep", (N, S))
    bf_rep = din("bf_rep", (N, S))
    wattn_rep = din("wattn_rep", (N, S))
    watsum = din("watsum", (N, 1))
    gm = din("gm", (128, T))
    bm = din("bm", (128, T))
    gi8 = din("gi8", (8, S))
    bi8 = din("bi8", (8, S))
    go8 = din("go8", (8, S))
    bo8 = din("bo8", (8, S))
    gu8 = din("gu8", (8, S))
    bu8 = din("bu8", (8, S))
    gc8 = din("gc8", (8, S))
    bc8 = din("bc8", (8, S))
    ones8 = din("ones8", (8, 1))
    ones32 = din("ones32", (N, 1))
    ones128 = din("ones128", (128, 1))
    onesr = din("onesr", (1, 128))
    wf = din("wf", (128, KT * S), BF16)        # [W_fh | W_fi]^T chunk
    wai = din("wai", (128, KT * S), BF16)      # W_ai^T chunk
    wmg = din("wmg", (128, 2 * H), BF16)       # W_merge^T in-chunk
    wiou = din("wiou", (128, KT * 3 * S), BF16)  # W_iou^T chunk (i|o|u cols)

    out_h = nc.dram_tensor("out_h", [8, S], F32, kind="ExternalOutput")
    out_c = nc.dram_tensor("out_c", [8, S], F32, kind="ExternalOutput")

    with tile.TileContext(nc) as tc:
        with (
            tc.tile_pool(name="sb", bufs=1) as sb,
            tc.tile_pool(name="ps", bufs=1, space="PSUM") as ps,
            tc.tile_pool(name="dram", bufs=1, space="DRAM") as dram,
        ):
            # ---- warmup collectives: zero deps, trigger at t~0 -----------
            # Contents are irrelevant; they pull communicator init and
            # per-kind first-collective software setup off the critical path.
            warm_in = dram.tile([1, 16], F32, name="warm_in")
            warm_out = dram.tile([8, 16], F32, name="warm_out")
            warm2_in = dram.tile([1, 16], F32, name="warm2_in")
            warm2_out = dram.tile([1, 16], F32, name="warm2_out")
            with tc.high_priority():
                nc.gpsimd.collective_compute(
                    "AllGather", ALU.bypass,
                    replica_groups=[list(range(NC))],
                    ins=[warm_in.opt()], outs=[warm_out.opt()])
                nc.gpsimd.collective_compute(
                    "AllReduce", ALU.add,
                    replica_groups=[list(range(NC))],
                    ins=[warm2_in.opt()], outs=[warm2_out.opt()])
                # single ACT table load for the whole kernel (ln+exp set)
                nc.scalar.add_instruction(mybir.InstLoadActFuncSet(
                    name=f"I-{nc.next_id()}", ins=[], outs=[],
                    act_func_set_id=LN_EXP_SET))

            # ------- small resident loads (scalar queue; SP is weights) ---
            def load(t_dram, shape, dt=F32):
                t_sb = sb.tile(shape, dt, name=t_dram.name + "_sb")
                nc.scalar.dma_start(t_sb[:], t_dram[:])
                return t_sb

            hT_sb = load(hT, [128, T, N], BF16)
            xT32_sb = load(xT32, [128, T, N], BF16)
            eT32_sb = load(eT32, [128, T, N], BF16)
            x1_sb = load(x1, [128, T], BF16)
            hTc_sb = load(hTc, [128, 2, N], BF16)
            cells_sb = load(cells_chunk, [N, S])
            wat_sb = load(wattn_rep, [N, S])
            wsum_sb = load(watsum, [N, 1])
            ones8_sb = load(ones8, [8, 1])
            ones32_sb = load(ones32, [N, 1])
            ones128_sb = load(ones128, [128, 1])
            onesr_sb = load(onesr, [1, 128])
            if not trivial_ln:
                gf_sb = load(gf_rep, [N, S])
                bf_sb = load(bf_rep, [N, S])
                gm_sb = load(gm, [128, T])
                bm_sb = load(bm, [128, T])
                gi_sb = load(gi8, [8, S])
                bi_sb = load(bi8, [8, S])
                go_sb = load(go8, [8, S])
                bo_sb = load(bo8, [8, S])
                gu_sb = load(gu8, [8, S])
                bu_sb = load(bu8, [8, S])
                gc_sb = load(gc8, [8, S])
                bc_sb = load(bc8, [8, S])

            eps1 = sb.tile([1, 1], F32, name="eps1")
            nc.vector.memset(eps1[:], EPS)
            epsN = sb.tile([N, 1], F32, name="epsN")
            nc.vector.memset(epsN[:], EPS)

            def recipA(out, in_):
                nc.vector.reciprocal_approx_fast(out=out, in_=in_)

            # ---------------- weight streaming DMAs (SP, ordered) ---------
            wai_sb = sb.tile([128, KT * S], BF16, name="wai_sb")
            wf_sb = sb.tile([128, KT * S], BF16, name="wf_sb")
            wiou_sb = sb.tile([128, KT * 3 * S], BF16, name="wiou_sb")
            wmg_sb = sb.tile([128, 2 * H], BF16, name="wmg_sb")

            wdmas = []
            for k in range(2):  # wai: 2 x 1MB
                wdmas.append(nc.sync.dma_start(
                    wai_sb[:, k * 4096:(k + 1) * 4096],
                    wai[:, k * 4096:(k + 1) * 4096]))
            for k in range(2):  # wf: 2 x 1MB
                wdmas.append(nc.sync.dma_start(
                    wf_sb[:, k * 4096:(k + 1) * 4096],
                    wf[:, k * 4096:(k + 1) * 4096]))
            for k in range(3):  # wiou x-half: 3 x 1MB
                wdmas.append(nc.sync.dma_start(
                    wiou_sb[:, k * 4096:(k + 1) * 4096],
                    wiou[:, k * 4096:(k + 1) * 4096]))
            wdmas.append(nc.sync.dma_start(wmg_sb[:], wmg[:]))  # 1MB
            for k in range(3, 6):  # wiou mh-half: 3 x 1MB
                wdmas.append(nc.sync.dma_start(
                    wiou_sb[:, k * 4096:(k + 1) * 4096],
                    wiou[:, k * 4096:(k + 1) * 4096]))
            # chain three-deep: keeps arrival order without strangling BW
            for i in range(3, len(wdmas)):
                add_dep_helper(wdmas[i].ins, wdmas[i - 3].ins, sync=True,
                               reason="weight DMA arrival order")

            # ---------------- attention: ai -> partial logits -------------
            ps_ai = ps.tile([N, S], F32, name="ps_ai", tag="pA")
            for kt in range(KT):
                act = hT_sb if kt < T else eT32_sb
                nc.tensor.matmul(ps_ai[:], act[:, kt % T, :],
                                 wai_sb[:, kt * S:(kt + 1) * S],
                                 start=(kt == 0), stop=(kt == KT - 1))
            # logits = sum_j wat * tanh(ai); tanh(w) = 2/(1+exp(-2w)) - 1
            # lg = 2 * sum(wat * r) - sum(wat),  r = 1/(1+exp(-2w))
            ai_e = sb.tile([N, S], F32, name="ai_e")
            nc.scalar.activation(ai_e[:], ps_ai[:], AF.Exp, scale=-2.0)
            ai_d = sb.tile([N, S], F32, name="ai_d")
            nc.vector.tensor_scalar_add(ai_d[:], ai_e[:], 1.0)
            ai_r = sb.tile([N, S], F32, name="ai_r")
            recipA(ai_r[:], ai_d[:])
            aw = sb.tile([N, S], F32, name="aw")
            asum = sb.tile([N, 1], F32, name="asum")
            nc.vector.scalar_tensor_tensor(aw[:], ai_r[:], 1.0, wat_sb[:],
                                           op0=ALU.mult, op1=ALU.mult,
                                           accum_out=asum[:])
            lg_sb = sb.tile([N, 1], F32, name="lg_sb")
            nc.vector.tensor_scalar(lg_sb[:], asum[:], 2.0, wsum_sb[:],
                                    op0=ALU.mult, op1=ALU.subtract)

            # ---------------- AG1: partial logits -------------------------
            ag1_in = dram.tile([1, N], F32, name="ag1_in")
            ag1_out_t = nc.dram_tensor("ag1_out", [8, N], F32,
                                       kind="Internal")
            nc.scalar.dma_start(
                ag1_in[0, :].rearrange("(p one) -> p one", one=1), lg_sb[:])
            nc.gpsimd.collective_compute(
                "AllGather", ALU.bypass,
                replica_groups=[list(range(NC))],
                ins=[ag1_in.opt()], outs=[ag1_out_t[:]])

            # ---------------- f_lin + per-child stats (speculative) -------
            ps_f = ps.tile([N, S], F32, name="ps_f", tag="pG")
            for kt in range(KT):
                act = hT_sb if kt < T else xT32_sb
                nc.tensor.matmul(ps_f[:], act[:, kt % T, :],
                                 wf_sb[:, kt * S:(kt + 1) * S],
                                 start=(kt == 0), stop=(kt == KT - 1))
            f_lin_sb = sb.tile([N, S], F32, name="f_lin_sb")
            fst2 = sb.tile([N, 2], F32, name="fst2")
            fsq_scr = sb.tile([N, S], F32, name="fsq_scr")
            nc.vector.tensor_copy(f_lin_sb[:], ps_f[:])
            nc.vector.tensor_reduce(fst2[:, 0:1], f_lin_sb[:],
                                    mybir.AxisListType.X, ALU.add)
            nc.vector.scalar_tensor_tensor(fsq_scr[:], f_lin_sb[:], 1.0,
                                           f_lin_sb[:], op0=ALU.mult,
                                           op1=ALU.mult,
                                           accum_out=fst2[:, 1:2])

            # speculative per-child merge projections (pre-attention):
            # M[p, t, n] = sum_in W_merge[t*128+p, in] * h[n, in], in-chunk
            ps_M = ps.tile([128, T, N], F32, name="ps_M", tag="pD")
            for t in range(T):
                for s in range(2):
                    nc.tensor.matmul(
                        ps_M[:, t, :],
                        wmg_sb[:, s * H + t * 128: s * H + (t + 1) * 128],
                        hTc_sb[:, s, :],
                        start=(s == 0), stop=(s == 1))

            # ---------------- iou x-half (speculative) --------------------
            ps_iou = ps.tile([1, 3 * S], F32, name="ps_iou", tag="pIOU")
            nslices = ((0, 512), (512, 768))

            def iou_mm(kt, start, stop):
                lhs = (x1_sb[:, kt:kt + 1] if kt < T
                       else mh_bf[:, kt - T:kt - T + 1])
                for c0, c1 in nslices:
                    nc.tensor.matmul(ps_iou[:, c0:c1],
                                     lhs, wiou_sb[:, kt * 768 + c0:
                                                  kt * 768 + c1],
                                     start=start, stop=stop)

            for kt in range(T):          # x half: runs as weights land
                iou_mm(kt, kt == 0, False)

            # ---------------- post-AG1: softmax + merge partials ----------
            ag1_sb = sb.tile([8, N], F32, name="ag1_sb")
            nc.sync.dma_start(ag1_sb[:], ag1_out_t[:])
            ps_l2r = ps.tile([1, N], F32, name="ps_l2r", tag="pB")
            nc.tensor.matmul(ps_l2r[:], ones8_sb[:], ag1_sb[:],
                             start=True, stop=True)
            exps_row = sb.tile([1, N], F32, name="exps_row")
            # softmax without max-subtraction or normalization: the scale
            # cancels inside the merge LayerNorm
            nc.scalar.activation(exps_row[:], ps_l2r[:], AF.Exp)
            ps_eb = ps.tile([128, N], F32, name="ps_eb", tag="pH")
            nc.tensor.matmul(ps_eb[:], onesr_sb[:], exps_row[:],
                             start=True, stop=True)

            # merge-linear partials: one multiply + one reduce
            exps_b = sb.tile([128, N], F32, name="exps_b")
            nc.vector.tensor_copy(exps_b[:], ps_eb[:])
            mp_sb = sb.tile([128, T], F32, name="mp_sb")
            mp_scr3 = sb.tile([128, T, N], F32, name="mp_scr3")
            eb3 = exps_b[:].rearrange("p (one n) -> p one n",
                                      one=1).to_broadcast((128, T, N))
            nc.vector.scalar_tensor_tensor(mp_scr3[:], ps_M[:], 1.0, eb3,
                                           op0=ALU.mult, op1=ALU.mult)
            nc.vector.tensor_reduce(mp_sb[:], mp_scr3[:],
                                    mybir.AxisListType.X, ALU.add)

            # ---------------- AR2: merge partials + f stats ---------------
            ar2_in = dram.tile([1, H + 2 * N], BF16, name="ar2_in")
            ar2_out = nc.dram_tensor("ar2_out", [1, H + 2 * N], BF16,
                                     kind="Internal")
            mp_bf = sb.tile([128, T], BF16, name="mp_bf")
            nc.vector.tensor_copy(mp_bf[:], mp_sb[:])
            nc.sync.dma_start(
                ar2_in[0, 0:H].rearrange("(p t) -> p t", p=128), mp_bf[:])
            fst_bf = sb.tile([N, 2], BF16, name="fst_bf")
            nc.vector.tensor_copy(fst_bf[:], fst2[:])
            nc.scalar.dma_start(
                ar2_in[0, H:H + 2 * N].rearrange("(p s) -> p s", s=2),
                fst_bf[:])
            nc.gpsimd.collective_compute(
                "AllReduce", ALU.add,
                replica_groups=[list(range(NC))],
                ins=[ar2_in.opt()], outs=[ar2_out[:]])

            # parallel readbacks on separate queues
            ml_sb = sb.tile([128, T], BF16, name="ml_sb")
            nc.sync.dma_start(
                ml_sb[:], ar2_out[0, 0:H].rearrange("(p t) -> p t", p=128))
            fst_t = sb.tile([N, 2], BF16, name="fst_t")
            nc.scalar.dma_start(
                fst_t[:], ar2_out[0, H:H + 2 * N].rearrange("(p s) -> p s",
                                                            s=2))

            # ---------------- merge-hidden LayerNorm + tanh ---------------
            st2 = sb.tile([128, 2], F32, name="st2")
            sq_scr = sb.tile([128, T], F32, name="sq_scr")
            nc.vector.tensor_reduce(st2[:, 0:1], ml_sb[:],
                                    mybir.AxisListType.X, ALU.add)
            nc.vector.scalar_tensor_tensor(sq_scr[:], ml_sb[:], 1.0,
                                           ml_sb[:], op0=ALU.mult,
                                           op1=ALU.mult,
                                           accum_out=st2[:, 1:2])
            ps_st = ps.tile([1, 2], F32, name="ps_st", tag="pB")
            nc.tensor.matmul(ps_st[:], ones128_sb[:], st2[:],
                             start=True, stop=True)
            mr = sb.tile([1, 2], F32, name="mr")
            var = sb.tile([1, 1], F32, name="mvar")
            lnv = sb.tile([1, 1], F32, name="mlnv")
            nc.vector.tensor_scalar_mul(mr[:], ps_st[:], INV_H)
            nc.vector.scalar_tensor_tensor(var[:], mr[:, 0:1], 1.0,
                                           mr[:, 0:1], op0=ALU.mult,
                                           op1=ALU.mult)
            nc.vector.tensor_sub(var[:], mr[:, 1:2], var[:])
            nc.scalar.activation(lnv[:], var[:], AF.Ln, bias=eps1[:])
            nc.scalar.activation(mr[:, 1:2], lnv[:], AF.Exp, scale=-0.5)
            ps_bc = ps.tile([128, 2], F32, name="ps_bc", tag="pH")
            nc.tensor.matmul(ps_bc[:], onesr_sb[:], mr[:],
                             start=True, stop=True)
            mrbm = sb.tile([128, 2], F32, name="mrbm")
            nc.vector.tensor_copy(mrbm[:], ps_bc[:])
            mh_e = sb.tile([128, T], F32, name="mh_e")
            if trivial_ln:
                # exp(2*(ml - m)*r) folded into ACT scale/bias:
                # scale = 2r, bias = -2*m*r  (per-partition broadcasts)
                s2r = sb.tile([128, 1], F32, name="s2r")
                nc.vector.tensor_scalar_mul(s2r[:], mrbm[:, 1:2], 2.0)
                b2mr = sb.tile([128, 1], F32, name="b2mr")
                nc.vector.scalar_tensor_tensor(b2mr[:], s2r[:], -1.0,
                                               mrbm[:, 0:1], op0=ALU.mult,
                                               op1=ALU.mult)
                nc.scalar.activation(mh_e[:], ml_sb[:], AF.Exp,
                                     bias=b2mr[:], scale=s2r[:])
            else:
                mh_t1 = sb.tile([128, T], F32, name="mh_t1")
                nc.vector.tensor_scalar(mh_t1[:], ml_sb[:], mrbm[:, 0:1],
                                        mrbm[:, 1:2], op0=ALU.subtract,
                                        op1=ALU.mult)
                nc.vector.scalar_tensor_tensor(mh_t1[:], mh_t1[:], 1.0,
                                               gm_sb[:], op0=ALU.mult,
                                               op1=ALU.mult)
                nc.vector.scalar_tensor_tensor(mh_t1[:], mh_t1[:], 1.0,
                                               bm_sb[:], op0=ALU.mult,
                                               op1=ALU.add)
                nc.scalar.activation(mh_e[:], mh_t1[:], AF.Exp, scale=2.0)
            nc.vector.tensor_scalar_add(mh_e[:], mh_e[:], 1.0)
            mh_r = sb.tile([128, T], F32, name="mh_r")
            recipA(mh_r[:], mh_e[:])
            mh_bf = sb.tile([128, T], BF16, name="mh_bf")
            nc.vector.tensor_scalar(mh_bf[:], mh_r[:], -2.0, 1.0,
                                    op0=ALU.mult, op1=ALU.add)

            # ---------------- iou mh-half (post-AR2 PE work) --------------
            for kt in range(T, KT):
                iou_mm(kt, False, kt == KT - 1)

            # ---------------- f gate + fc = sum_n f*cells -----------------
            fmr = sb.tile([N, 2], F32, name="fmr")
            flnv = sb.tile([N, 1], F32, name="flnv")
            fvar = sb.tile([N, 1], F32, name="fvar")
            nc.vector.tensor_scalar_mul(fmr[:], fst_t[:], INV_H)
            nc.vector.scalar_tensor_tensor(fvar[:], fmr[:, 0:1], 1.0,
                                           fmr[:, 0:1], op0=ALU.mult,
                                           op1=ALU.mult)
            nc.vector.tensor_sub(fvar[:], fmr[:, 1:2], fvar[:])
            nc.scalar.activation(flnv[:], fvar[:], AF.Ln, bias=epsN[:])
            nc.scalar.activation(fmr[:, 1:2], flnv[:], AF.Exp, scale=-0.5)
            f_e = sb.tile([N, S], F32, name="f_e")
            if trivial_ln:
                # sig((x-m)*r): exp(-(x-m)*r) via scale=-r, bias=m*r
                fnr = sb.tile([N, 1], F32, name="fnr")
                nc.vector.tensor_scalar_mul(fnr[:], fmr[:, 1:2], -1.0)
                fmrb = sb.tile([N, 1], F32, name="fmrb")
                nc.vector.scalar_tensor_tensor(fmrb[:], fnr[:], -1.0,
                                               fmr[:, 0:1], op0=ALU.mult,
                                               op1=ALU.mult)
                nc.scalar.activation(f_e[:], f_lin_sb[:], AF.Exp,
                                     bias=fmrb[:], scale=fnr[:])
            else:
                ft = sb.tile([N, S], F32, name="ft")
                nc.vector.tensor_scalar(ft[:], f_lin_sb[:], fmr[:, 0:1],
                                        fmr[:, 1:2], op0=ALU.subtract,
                                        op1=ALU.mult)
                nc.vector.scalar_tensor_tensor(ft[:], ft[:], 1.0, gf_sb[:],
                                               op0=ALU.mult, op1=ALU.mult)
                nc.vector.scalar_tensor_tensor(ft[:], ft[:], 1.0, bf_sb[:],
                                               op0=ALU.mult, op1=ALU.add)
                nc.scalar.activation(f_e[:], ft[:], AF.Exp, scale=-1.0)
            nc.vector.tensor_scalar_add(f_e[:], f_e[:], 1.0)
            f_r = sb.tile([N, S], F32, name="f_r")
            recipA(f_r[:], f_e[:])
            fprod = sb.tile([N, S], F32, name="fprod")
            nc.vector.scalar_tensor_tensor(fprod[:], f_r[:], 1.0,
                                           cells_sb[:], op0=ALU.mult,
                                           op1=ALU.mult)
            ps_fc = ps.tile([1, S], F32, name="ps_fc", tag="pC")
            nc.tensor.matmul(ps_fc[:], ones32_sb[:], fprod[:],
                             start=True, stop=True)

            # ---------------- AG3: iou chunk + fc chunk -------------------
            ag3_in = dram.tile([1, 4 * S], BF16, name="ag3_in")
            ag3_out = nc.dram_tensor("ag3_out", [8, 4 * S], BF16,
                                     kind="Internal")
            fc_bf = sb.tile([1, S], BF16, name="fc_bf")
            nc.scalar.activation(fc_bf[:], ps_fc[:], AF.Copy)
            nc.scalar.dma_start(ag3_in[0, 3 * S:4 * S].rearrange(
                "(a f) -> a f", a=1), fc_bf[:])
            ag3_stage = sb.tile([1, 3 * S], BF16, name="ag3_stage")
            nc.vector.tensor_copy(ag3_stage[:, 0:512], ps_iou[:, 0:512])
            nc.scalar.activation(ag3_stage[:, 512:3 * S],
                                 ps_iou[:, 512:3 * S], AF.Copy)
            nc.sync.dma_start(ag3_in[0, 0:3 * S].rearrange(
                "(a f) -> a f", a=1), ag3_stage[:])
            nc.gpsimd.collective_compute(
                "AllGather", ALU.bypass,
                replica_groups=[list(range(NC))],
                ins=[ag3_in.opt()], outs=[ag3_out[:]])

            # single contiguous readback: [8, 1024] chunk-major
            ag3_sb = sb.tile([8, 4 * S], BF16, name="ag3_sb")
            nc.sync.dma_start(ag3_sb[:], ag3_out[:])
            i_l = ag3_sb[:, 0 * S:1 * S]
            o_l = ag3_sb[:, 1 * S:2 * S]
            u_l = ag3_sb[:, 2 * S:3 * S]
            fc_v = ag3_sb[:, 3 * S:4 * S]

            # LayerNorm stats for i/o/u: sums in one DVE reduce, squared
            # sums on the ACT engine (Square + accumulate), in parallel
            st6 = sb.tile([8, 6], F32, name="st6")
            iou3 = ag3_sb[:, 0:3 * S].rearrange("p (g f) -> p g f", g=3)
            nc.vector.tensor_reduce(st6[:, 0:3], iou3,
                                    mybir.AxisListType.X, ALU.add)
            sqa_scr = sb.tile([8, S], F32, name="sqa_scr")
            for v, vec in enumerate((i_l, o_l, u_l)):
                nc.scalar.activation(sqa_scr[:], vec, AF.Square,
                                     accum_out=st6[:, 3 + v:4 + v])
            ps_st6 = ps.tile([1, 6], F32, name="ps_st6", tag="pA")
            nc.tensor.matmul(ps_st6[:], ones8_sb[:], st6[:],
                             start=True, stop=True)
            mr6 = sb.tile([1, 6], F32, name="mr6")
            var3 = sb.tile([1, 3], F32, name="var3")
            lnv3 = sb.tile([1, 3], F32, name="lnv3")
            nc.vector.tensor_scalar_mul(mr6[:], ps_st6[:], INV_H)
            nc.vector.scalar_tensor_tensor(var3[:], mr6[:, 0:3], 1.0,
                                           mr6[:, 0:3], op0=ALU.mult,
                                           op1=ALU.mult)
            nc.vector.tensor_sub(var3[:], mr6[:, 3:6], var3[:])
            nc.scalar.activation(lnv3[:], var3[:], AF.Ln, bias=eps1[:])
            nc.scalar.activation(mr6[:, 3:6], lnv3[:], AF.Exp, scale=-0.5)
            ps_b6 = ps.tile([8, 6], F32, name="ps_b6", tag="pB")
            nc.tensor.matmul(ps_b6[:], onesr_sb[:, 0:8], mr6[:],
                             start=True, stop=True)
            mrb6 = sb.tile([8, 6], F32, name="mrb6")
            nc.vector.tensor_copy(mrb6[:], ps_b6[:])

            if trivial_ln:
                # negated / doubled per-partition scale-bias precomputes
                nr3 = sb.tile([8, 3], F32, name="nr3")
                nc.vector.tensor_scalar_mul(nr3[:], mrb6[:, 3:6], -1.0)
                mr3 = sb.tile([8, 3], F32, name="mr3")
                nc.vector.scalar_tensor_tensor(mr3[:], mrb6[:, 0:3], 1.0,
                                               mrb6[:, 3:6], op0=ALU.mult,
                                               op1=ALU.mult)
                p2ru = sb.tile([8, 1], F32, name="p2ru")
                nc.vector.tensor_scalar_mul(p2ru[:], mrb6[:, 5:6], 2.0)
                n2mru = sb.tile([8, 1], F32, name="n2mru")
                nc.vector.tensor_scalar_mul(n2mru[:], mr3[:, 2:3], -2.0)

                e3 = sb.tile([8, 3 * S], F32, name="e3")
                nc.scalar.activation(e3[:, 0:S], i_l, AF.Exp,
                                     bias=mr3[:, 0:1], scale=nr3[:, 0:1])
                nc.scalar.activation(e3[:, S:2 * S], o_l, AF.Exp,
                                     bias=mr3[:, 1:2], scale=nr3[:, 1:2])
                nc.scalar.activation(e3[:, 2 * S:3 * S], u_l, AF.Exp,
                                     bias=n2mru[:], scale=p2ru[:])
                nc.vector.tensor_scalar_add(e3[:], e3[:], 1.0)
                r3g = sb.tile([8, 3 * S], F32, name="r3g")
                recipA(r3g[:], e3[:])
                i_g = r3g[:, 0:S]
                o_g = r3g[:, S:2 * S]
                u_r3 = r3g[:, 2 * S:3 * S]
            else:
                def gate_ln(vec, v, g_t, b_t, nm):
                    t1 = sb.tile([8, S], F32, name=nm + "_t1")
                    nc.vector.tensor_scalar(t1[:], vec,
                                            mrb6[:, v:v + 1],
                                            mrb6[:, 3 + v:4 + v],
                                            op0=ALU.subtract, op1=ALU.mult)
                    nc.vector.scalar_tensor_tensor(t1[:], t1[:], 1.0,
                                                   g_t[:], op0=ALU.mult,
                                                   op1=ALU.mult)
                    nc.vector.scalar_tensor_tensor(t1[:], t1[:], 1.0,
                                                   b_t[:], op0=ALU.mult,
                                                   op1=ALU.add)
                    return t1

                yi = gate_ln(i_l, 0, gi_sb, bi_sb, "yi")
                yo = gate_ln(o_l, 1, go_sb, bo_sb, "yo")
                yu = gate_ln(u_l, 2, gu_sb, bu_sb, "yu")

                def sig(y, nm):
                    e = sb.tile([8, S], F32, name=nm + "_e")
                    nc.scalar.activation(e[:], y[:], AF.Exp, scale=-1.0)
                    nc.vector.tensor_scalar_add(e[:], e[:], 1.0)
                    r = sb.tile([8, S], F32, name=nm + "_r")
                    recipA(r[:], e[:])
                    return r

                i_g = sig(yi, "ig")
                o_g = sig(yo, "og")
                u_e = sb.tile([8, S], F32, name="u_e")
                nc.scalar.activation(u_e[:], yu[:], AF.Exp, scale=2.0)

            if trivial_ln:
                u_rv = u_r3
                i_gv, o_gv = i_g, o_g
            else:
                nc.vector.tensor_scalar_add(u_e[:], u_e[:], 1.0)
                u_r = sb.tile([8, S], F32, name="u_r")
                recipA(u_r[:], u_e[:])
                u_rv = u_r[:]
                i_gv, o_gv = i_g[:], o_g[:]
            # cell_lin = i*u + fc = i - 2*i*ru + fc
            iru = sb.tile([8, S], F32, name="iru")
            nc.vector.scalar_tensor_tensor(iru[:], u_rv, 1.0, i_gv,
                                           op0=ALU.mult, op1=ALU.mult)
            cell_lin = sb.tile([8, S], F32, name="cell_lin")
            nc.vector.scalar_tensor_tensor(cell_lin[:], iru[:], -2.0,
                                           i_gv, op0=ALU.mult,
                                           op1=ALU.add)
            nc.vector.scalar_tensor_tensor(cell_lin[:], cell_lin[:], 1.0,
                                           fc_v, op0=ALU.mult, op1=ALU.add)

            # cell LayerNorm
            cst = sb.tile([8, 2], F32, name="cst")
            csq = sb.tile([8, S], F32, name="csq")
            nc.vector.tensor_reduce(cst[:, 0:1], cell_lin[:],
                                    mybir.AxisListType.X, ALU.add)
            nc.scalar.activation(csq[:], cell_lin[:], AF.Square,
                                 accum_out=cst[:, 1:2])
            ps_cst = ps.tile([1, 2], F32, name="ps_cst", tag="pC")
            nc.tensor.matmul(ps_cst[:], ones8_sb[:], cst[:],
                             start=True, stop=True)
            cmr = sb.tile([1, 2], F32, name="cmr")
            cvar = sb.tile([1, 1], F32, name="cvar")
            clnv = sb.tile([1, 1], F32, name="clnv")
            nc.vector.tensor_scalar_mul(cmr[:], ps_cst[:], INV_H)
            nc.vector.scalar_tensor_tensor(cvar[:], cmr[:, 0:1], 1.0,
                                           cmr[:, 0:1], op0=ALU.mult,
                                           op1=ALU.mult)
            nc.vector.tensor_sub(cvar[:], cmr[:, 1:2], cvar[:])
            nc.scalar.activation(clnv[:], cvar[:], AF.Ln, bias=eps1[:])
            nc.scalar.activation(cmr[:, 1:2], clnv[:], AF.Exp, scale=-0.5)
            ps_cb = ps.tile([8, 2], F32, name="ps_cb", tag="pD")
            nc.tensor.matmul(ps_cb[:], onesr_sb[:, 0:8], cmr[:],
                             start=True, stop=True)
            mrbc = sb.tile([8, 2], F32, name="mrbc")
            nc.vector.tensor_copy(mrbc[:], ps_cb[:])
            new_c = sb.tile([8, S], F32, name="new_c")
            t_e = sb.tile([8, S], F32, name="t_e")
            if trivial_ln:
                nc.vector.tensor_scalar(new_c[:], cell_lin[:],
                                        mrbc[:, 0:1], mrbc[:, 1:2],
                                        op0=ALU.subtract, op1=ALU.mult)
                # tanh arg: exp(2*(cell-m)*r) via scale=2r, bias=-2mr
                c2r = sb.tile([8, 1], F32, name="c2r")
                nc.vector.tensor_scalar_mul(c2r[:], mrbc[:, 1:2], 2.0)
                cn2mr = sb.tile([8, 1], F32, name="cn2mr")
                nc.vector.scalar_tensor_tensor(cn2mr[:], c2r[:], -1.0,
                                               mrbc[:, 0:1], op0=ALU.mult,
                                               op1=ALU.mult)
                nc.scalar.activation(t_e[:], cell_lin[:], AF.Exp,
                                     bias=cn2mr[:], scale=c2r[:])
            else:
                nc.vector.tensor_scalar(new_c[:], cell_lin[:],
                                        mrbc[:, 0:1], mrbc[:, 1:2],
                                        op0=ALU.subtract, op1=ALU.mult)
                nc.vector.scalar_tensor_tensor(new_c[:], new_c[:], 1.0,
                                               gc_sb[:], op0=ALU.mult,
                                               op1=ALU.mult)
                nc.vector.scalar_tensor_tensor(new_c[:], new_c[:], 1.0,
                                               bc_sb[:], op0=ALU.mult,
                                               op1=ALU.add)
                nc.scalar.activation(t_e[:], new_c[:], AF.Exp, scale=2.0)

            # new_h = o * tanh(.) = o - 2*o*rt
            nc.vector.tensor_scalar_add(t_e[:], t_e[:], 1.0)
            t_r = sb.tile([8, S], F32, name="t_r")
            recipA(t_r[:], t_e[:])
            ort = sb.tile([8, S], F32, name="ort")
            nc.vector.scalar_tensor_tensor(ort[:], t_r[:], 1.0, o_gv,
                                           op0=ALU.mult, op1=ALU.mult)
            new_h = sb.tile([8, S], F32, name="new_h")
            nc.vector.scalar_tensor_tensor(new_h[:], ort[:], -2.0, o_gv,
                                           op0=ALU.mult, op1=ALU.add)

            nc.sync.dma_start(out_c[:], new_c[:])
            nc.scalar.dma_start(out_h[:], new_h[:])

    nc.compile()
    return nc


def _tmaj(v):
    """[2048] vector -> [128,16] t-major sbuf image (sb[p,t] = v[t*128+p])."""
    return np.ascontiguousarray(v.reshape(T, 128).T)


def _ktiles(wT, cols):
    """wT: [K_in, out_cols] -> [128, (K_in/128)*cols] partition-major pack."""
    k_in = wT.shape[0]
    return np.ascontiguousarray(
        wT.reshape(k_in // 128, 128, cols).transpose(1, 0, 2).reshape(
            128, (k_in // 128) * cols))


def kernel(input, hiddens, cells, external,
           W_ai, W_attn, W_merge, W_iou, W_fi, W_fh,
           g_merge, b_merge, g_f, b_f, g_i, b_i, g_o, b_o, g_u, b_u,
           g_c, b_c):
    f32 = np.float32
    gs = [np.asarray(g, f32) for g in
          (g_merge, g_f, g_i, g_o, g_u, g_c)]
    bs = [np.asarray(b, f32) for b in
          (b_merge, b_f, b_i, b_o, b_u, b_c)]
    trivial = (all(np.all(g == 1.0) for g in gs)
               and all(np.all(b == 0.0) for b in bs))
    key = ("nc", trivial)
    if key not in _CACHE:
        _CACHE[key] = _build(trivial)
    nc = _CACHE[key]

    input = np.asarray(input, f32)
    hiddens = np.asarray(hiddens, f32)
    cells = np.asarray(cells, f32)
    external = np.asarray(external, f32)

    hTt = _ktiles(np.ascontiguousarray(hiddens.T), N).astype(NPBF)
    xT32 = _ktiles(np.tile(input[:, None], (1, N)), N).astype(NPBF)
    eT32 = _ktiles(np.tile(external[:, None], (1, N)), N).astype(NPBF)
    x1 = _tmaj(input).astype(NPBF)

    com = {
        "hT": hTt, "xT32": xT32, "eT32": eT32, "x1": x1,
        "gm": _tmaj(gs[0]), "bm": _tmaj(bs[0]),
        "gi8": gs[2].reshape(8, S), "bi8": bs[2].reshape(8, S),
        "go8": gs[3].reshape(8, S), "bo8": bs[3].reshape(8, S),
        "gu8": gs[4].reshape(8, S), "bu8": bs[4].reshape(8, S),
        "gc8": gs[5].reshape(8, S), "bc8": bs[5].reshape(8, S),
        "ones8": np.ones((8, 1), f32), "ones32": np.ones((N, 1), f32),
        "ones128": np.ones((128, 1), f32),
        "onesr": np.ones((1, 128), f32),
    }

    Wf_cat = np.concatenate([W_fh, W_fi], axis=1)              # [H, 4096]
    in_maps = []
    for c in range(NC):
        r = slice(c * S, (c + 1) * S)
        iou_rows = np.concatenate(
            [W_iou[g * H + c * S:g * H + (c + 1) * S, :] for g in range(3)],
            axis=0)                                            # [768, 4096]
        m = dict(com)
        m.update({
            "hTc": np.ascontiguousarray(
                hiddens.T[c * S:(c + 1) * S].reshape(2, 128, N)
                .transpose(1, 0, 2).reshape(128, 2 * N)).astype(NPBF),
            "cells_chunk": np.ascontiguousarray(cells[:, r]),
            "gf_rep": np.tile(gs[1][r], (N, 1)),
            "bf_rep": np.tile(bs[1][r], (N, 1)),
            "wattn_rep": np.tile(np.asarray(W_attn, f32)[0, r], (N, 1)),
            "watsum": np.full((N, 1), np.asarray(W_attn, f32)[0, r].sum(),
                              f32),
            "wf": _ktiles(np.ascontiguousarray(Wf_cat[r].T), S).astype(NPBF),
            "wai": _ktiles(np.ascontiguousarray(W_ai[r].T), S).astype(NPBF),
            "wmg": _ktiles(np.ascontiguousarray(W_merge[:, r].T),
                           H).astype(NPBF),
            "wiou": _ktiles(np.ascontiguousarray(iou_rows.T),
                            3 * S).astype(NPBF),
        })
        in_maps.append({k: (np.ascontiguousarray(v) if v.dtype == NPBF
                            else np.ascontiguousarray(v, f32))
                        for k, v in m.items()})

    res = run_bass_kernel_spmd(nc, in_maps, core_ids=list(range(NC)))
    _CACHE["last_results"] = res
    r0 = res.results[0]
    new_h = r0["out_h"].reshape(H).astype(f32)
    new_c = r0["out_c"].reshape(H).astype(f32)
    return new_h, new_c


# revision 13
# speedup vs baseline: 1.0635x; 1.0635x over previous
"""AttentiveChildSumTreeLSTMCell on 8 Trainium2 NeuronCores.

Tensor-parallel: column-parallel f/attention/iou linears (hidden dim sharded
8 ways), row-parallel merge linear.  Collectives: two zero-dependency warmup
AllGathers (absorb communicator init + first-collective software setup),
AllGather of partial attention logits, AllReduce of merge-linear partials +
f LayerNorm stats, AllGather of iou/forget*cell chunks.  Matmul operands
are bf16; accumulation and all norm/gate math stays fp32.

All activations use a single ACT table set (ln+exp, loaded once): sigmoid
and tanh are computed via exp + DVE fast-reciprocal, LayerNorm rstd via
exp(-0.5*ln(var+eps)).  When all LayerNorm gains are 1 and biases 0 (the
common case, verified at runtime), the (x-mean)*rstd normalization is
folded into the exp activation's per-partition scale/bias operands.  The
gpsimd queue carries only collective triggers; the final gate math runs in
a [8, 256] chunk layout read straight from the AllGather result.
"""

import sys

for _p in ("/opt/trn_rl_repo",):
    if _p not in sys.path:
        sys.path.insert(0, _p)

import ml_dtypes
import numpy as np

import concourse.bacc as bacc
import concourse.mybir as mybir
import concourse.tile as tile
from concourse.bass_utils import run_bass_kernel_spmd
from concourse.tile_rust import add_dep_helper

F32 = mybir.dt.float32
BF16 = mybir.dt.bfloat16
AF = mybir.ActivationFunctionType
ALU = mybir.AluOpType
NPBF = ml_dtypes.bfloat16

H = 2048
N = 32
NC = 8
S = H // NC           # 256: per-core chunk of every sharded dim
T = H // 128          # 16 tiles of 128 along a 2048 dim
KT = 32               # K-tiles along the 4096 contraction dims
EPS = 1e-5
INV_H = 1.0 / H

# index of the ln+exp activation-function set in act_info.json
LN_EXP_SET = 6

_CACHE = {}


def _build(trivial_ln):
    nc = bacc.Bacc(None, target_bir_lowering=False, debug=False, num_devices=NC)

    def din(name, shape, dt=F32):
        return nc.dram_tensor(name, list(shape), dt, kind="ExternalInput")

    # ---- per-core DRAM inputs (SPMD: same shapes on every core) ----
    hT = din("hT", (128, T * N), BF16)
    xT32 = din("xT32", (128, T * N), BF16)
    eT32 = din("eT32", (128, T * N), BF16)
    x1 = din("x1", (128, T), BF16)
    hTc = din("hTc", (128, 2 * N), BF16)
    cells_chunk = din("cells_chunk", (N, S))
    gf_rep = din("gf_rep", (N, S))
    bf_rep = din("bf_rep", (N, S))
    wattn_rep = din("wattn_rep", (N, S))
    watsum = din("watsum", (N, 1))
    gm = din("gm", (128, T))
    bm = din("bm", (128, T))
    gi8 = din("gi8", (8, S))
    bi8 = din("bi8", (8, S))
    go8 = din("go8", (8, S))
    bo8 = din("bo8", (8, S))
    gu8 = din("gu8", (8, S))
    bu8 = din("bu8", (8, S))
    gc8 = din("gc8", (8, S))
    bc8 = din("bc8", (8, S))
    ones8 = din("ones8", (8, 1))
    ones32 = din("ones32", (N, 1))
    ones128 = din("ones128", (128, 1))
    onesr = din("onesr", (1, 128))
    wf = din("wf", (128, KT * S), BF16)        # [W_fh | W_fi]^T chunk
    wai = din("wai", (128, KT * S), BF16)      # W_ai^T chunk
    wmg = din("wmg", (128, 2 * H), BF16)       # W_merge^T in-chunk
    wiou = din("wiou", (128, KT * 3 * S), BF16)  # W_iou^T chunk (i|o|u cols)

    out_h = nc.dram_tensor("out_h", [8, S], F32, kind="ExternalOutput")
    out_c = nc.dram_tensor("out_c", [8, S], F32, kind="ExternalOutput")

    with tile.TileContext(nc) as tc:
        with (
            tc.tile_pool(name="sb", bufs=1) as sb,
            tc.tile_pool(name="ps", bufs=1, space="PSUM") as ps,
            tc.tile_pool(name="dram", bufs=1, space="DRAM") as dram,
        ):
            # ---- warmup collectives: zero deps, trigger at t~0 -----------
            # Contents are irrelevant; they pull communicator init and
            # per-kind first-collective software setup off the critical path.
            warm_in = dram.tile([1, 16], F32, name="warm_in")
            warm_out = dram.tile([8, 16], F32, name="warm_out")
            warm2_in = dram.tile([1, 16], F32, name="warm2_in")
            warm2_out = dram.tile([1, 16], F32, name="warm2_out")
            with tc.high_priority():
                nc.gpsimd.collective_compute(
                    "AllGather", ALU.bypass,
                    replica_groups=[list(range(NC))],
                    ins=[warm_in.opt()], outs=[warm_out.opt()])
                nc.gpsimd.collective_compute(
                    "AllReduce", ALU.add,
                    replica_groups=[list(range(NC))],
                    ins=[warm2_in.opt()], outs=[warm2_out.opt()])
                # single ACT table load for the whole kernel (ln+exp set)
                nc.scalar.add_instruction(mybir.InstLoadActFuncSet(
                    name=f"I-{nc.next_id()}", ins=[], outs=[],
                    act_func_set_id=LN_EXP_SET))

            # ------- small resident loads (scalar queue; SP is weights) ---
            def load(t_dram, shape, dt=F32):
                t_sb = sb.tile(shape, dt, name=t_dram.name + "_sb")
                nc.scalar.dma_start(t_sb[:], t_dram[:])
                return t_sb

            hT_sb = load(hT, [128, T, N], BF16)
            xT32_sb = load(xT32, [128, T, N], BF16)
            eT32_sb = load(eT32, [128, T, N], BF16)
            x1_sb = load(x1, [128, T], BF16)
            hTc_sb = load(hTc, [128, 2, N], BF16)
            cells_sb = load(cells_chunk, [N, S])
            wat_sb = load(wattn_rep, [N, S])
            wsum_sb = load(watsum, [N, 1])
            ones8_sb = load(ones8, [8, 1])
            ones32_sb = load(ones32, [N, 1])
            ones128_sb = load(ones128, [128, 1])
            onesr_sb = load(onesr, [1, 128])
            if not trivial_ln:
                gf_sb = load(gf_rep, [N, S])
                bf_sb = load(bf_rep, [N, S])
                gm_sb = load(gm, [128, T])
                bm_sb = load(bm, [128, T])
                gi_sb = load(gi8, [8, S])
                bi_sb = load(bi8, [8, S])
                go_sb = load(go8, [8, S])
                bo_sb = load(bo8, [8, S])
                gu_sb = load(gu8, [8, S])
                bu_sb = load(bu8, [8, S])
                gc_sb = load(gc8, [8, S])
                bc_sb = load(bc8, [8, S])

            eps1 = sb.tile([1, 1], F32, name="eps1")
            nc.vector.memset(eps1[:], EPS)
            epsN = sb.tile([N, 1], F32, name="epsN")
            nc.vector.memset(epsN[:], EPS)

            def recipA(out, in_):
                nc.vector.reciprocal_approx_fast(out=out, in_=in_)

            # ---------------- weight streaming DMAs (SP, ordered) ---------
            wai_sb = sb.tile([128, KT * S], BF16, name="wai_sb")
            wf_sb = sb.tile([128, KT * S], BF16, name="wf_sb")
            wiou_sb = sb.tile([128, KT * 3 * S], BF16, name="wiou_sb")
            wmg_sb = sb.tile([128, 2 * H], BF16, name="wmg_sb")

            wdmas = []
            for k in range(2):  # wai: 2 x 1MB
                wdmas.append(nc.sync.dma_start(
                    wai_sb[:, k * 4096:(k + 1) * 4096],
                    wai[:, k * 4096:(k + 1) * 4096]))
            for k in range(2):  # wf: 2 x 1MB
                wdmas.append(nc.sync.dma_start(
                    wf_sb[:, k * 4096:(k + 1) * 4096],
                    wf[:, k * 4096:(k + 1) * 4096]))
            for k in range(3):  # wiou x-half: 3 x 1MB
                wdmas.append(nc.sync.dma_start(
                    wiou_sb[:, k * 4096:(k + 1) * 4096],
                    wiou[:, k * 4096:(k + 1) * 4096]))
            wdmas.append(nc.sync.dma_start(wmg_sb[:], wmg[:]))  # 1MB
            for k in range(3, 6):  # wiou mh-half: 3 x 1MB
                wdmas.append(nc.sync.dma_start(
                    wiou_sb[:, k * 4096:(k + 1) * 4096],
                    wiou[:, k * 4096:(k + 1) * 4096]))
            # chain three-deep: keeps arrival order without strangling BW
            for i in range(3, len(wdmas)):
                add_dep_helper(wdmas[i].ins, wdmas[i - 3].ins, sync=True,
                               reason="weight DMA arrival order")

            # ---------------- attention: ai -> partial logits -------------
            ps_ai = ps.tile([N, S], F32, name="ps_ai", tag="pA")
            for kt in range(KT):
                act = hT_sb if kt < T else eT32_sb
                nc.tensor.matmul(ps_ai[:], act[:, kt % T, :],
                                 wai_sb[:, kt * S:(kt + 1) * S],
                                 start=(kt == 0), stop=(kt == KT - 1))
            # logits = sum_j wat * tanh(ai); tanh(w) = 2/(1+exp(-2w)) - 1
            # lg = 2 * sum(wat * r) - sum(wat),  r = 1/(1+exp(-2w))
            ai_e = sb.tile([N, S], F32, name="ai_e")
            nc.scalar.activation(ai_e[:], ps_ai[:], AF.Exp, scale=-2.0)
            ai_d = sb.tile([N, S], F32, name="ai_d")
            nc.vector.tensor_scalar_add(ai_d[:], ai_e[:], 1.0)
            ai_r = sb.tile([N, S], F32, name="ai_r")
            recipA(ai_r[:], ai_d[:])
            aw = sb.tile([N, S], F32, name="aw")
            asum = sb.tile([N, 1], F32, name="asum")
            nc.vector.scalar_tensor_tensor(aw[:], ai_r[:], 1.0, wat_sb[:],
                                           op0=ALU.mult, op1=ALU.mult,
                                           accum_out=asum[:])
            lg_sb = sb.tile([N, 1], F32, name="lg_sb")
            nc.vector.tensor_scalar(lg_sb[:], asum[:], 2.0, wsum_sb[:],
                                    op0=ALU.mult, op1=ALU.subtract)

            # ---------------- AG1: partial logits -------------------------
            ag1_in = dram.tile([1, N], F32, name="ag1_in")
            ag1_out_t = nc.dram_tensor("ag1_out", [8, N], F32,
                                       kind="Internal", addr_space="Shared")
            nc.scalar.dma_start(
                ag1_in[0, :].rearrange("(p one) -> p one", one=1), lg_sb[:])
            nc.gpsimd.collective_compute(
                "AllGather", ALU.bypass,
                replica_groups=[list(range(NC))],
                ins=[ag1_in.opt()], outs=[ag1_out_t[:]])

            # ---------------- f_lin + per-child stats (speculative) -------
            ps_f = ps.tile([N, S], F32, name="ps_f", tag="pG")
            for kt in range(KT):
                act = hT_sb if kt < T else xT32_sb
                nc.tensor.matmul(ps_f[:], act[:, kt % T, :],
                                 wf_sb[:, kt * S:(kt + 1) * S],
                                 start=(kt == 0), stop=(kt == KT - 1))
            f_lin_sb = sb.tile([N, S], F32, name="f_lin_sb")
            fst2 = sb.tile([N, 2], F32, name="fst2")
            fsq_scr = sb.tile([N, S], F32, name="fsq_scr")
            nc.vector.tensor_copy(f_lin_sb[:], ps_f[:])
            nc.vector.tensor_reduce(fst2[:, 0:1], f_lin_sb[:],
                                    mybir.AxisListType.X, ALU.add)
            nc.vector.scalar_tensor_tensor(fsq_scr[:], f_lin_sb[:], 1.0,
                                           f_lin_sb[:], op0=ALU.mult,
                                           op1=ALU.mult,
                                           accum_out=fst2[:, 1:2])

            # speculative per-child merge projections (pre-attention):
            # M[p, t, n] = sum_in W_merge[t*128+p, in] * h[n, in], in-chunk
            ps_M = ps.tile([128, T, N], F32, name="ps_M", tag="pD")
            for t in range(T):
                for s in range(2):
                    nc.tensor.matmul(
                        ps_M[:, t, :],
                        wmg_sb[:, s * H + t * 128: s * H + (t + 1) * 128],
                        hTc_sb[:, s, :],
                        start=(s == 0), stop=(s == 1))

            # ---------------- iou x-half (speculative) --------------------
            ps_iou = ps.tile([1, 3 * S], F32, name="ps_iou", tag="pIOU")
            nslices = ((0, 512), (512, 768))

            def iou_mm(kt, start, stop):
                lhs = (x1_sb[:, kt:kt + 1] if kt < T
                       else mh_bf[:, kt - T:kt - T + 1])
                for c0, c1 in nslices:
                    nc.tensor.matmul(ps_iou[:, c0:c1],
                                     lhs, wiou_sb[:, kt * 768 + c0:
                                                  kt * 768 + c1],
                                     start=start, stop=stop)

            for kt in range(T):          # x half: runs as weights land
                iou_mm(kt, kt == 0, False)

            # ---------------- post-AG1: softmax + merge partials ----------
            ag1_sb = sb.tile([8, N], F32, name="ag1_sb")
            nc.sync.dma_start(ag1_sb[:], ag1_out_t[:])
            ps_l2r = ps.tile([1, N], F32, name="ps_l2r", tag="pB")
            nc.tensor.matmul(ps_l2r[:], ones8_sb[:], ag1_sb[:],
                             start=True, stop=True)
            exps_row = sb.tile([1, N], F32, name="exps_row")
            # softmax without max-subtraction or normalization: the scale
            # cancels inside the merge LayerNorm
            nc.scalar.activation(exps_row[:], ps_l2r[:], AF.Exp)
            ps_eb = ps.tile([128, N], F32, name="ps_eb", tag="pH")
            nc.tensor.matmul(ps_eb[:], onesr_sb[:], exps_row[:],
                             start=True, stop=True)

            # merge-linear partials: one multiply + one reduce
            exps_b = sb.tile([128, N], F32, name="exps_b")
            nc.vector.tensor_copy(exps_b[:], ps_eb[:])
            mp_sb = sb.tile([128, T], F32, name="mp_sb")
            mp_scr3 = sb.tile([128, T, N], F32, name="mp_scr3")
            eb3 = exps_b[:].rearrange("p (one n) -> p one n",
                                      one=1).to_broadcast((128, T, N))
            nc.vector.scalar_tensor_tensor(mp_scr3[:], ps_M[:], 1.0, eb3,
                                           op0=ALU.mult, op1=ALU.mult)
            nc.vector.tensor_reduce(mp_sb[:], mp_scr3[:],
                                    mybir.AxisListType.X, ALU.add)

            # ---------------- AR2: merge partials + f stats ---------------
            ar2_in = dram.tile([1, H + 2 * N], BF16, name="ar2_in")
            ar2_out = nc.dram_tensor("ar2_out", [1, H + 2 * N], BF16,
                                     kind="Internal", addr_space="Shared")
            mp_bf = sb.tile([128, T], BF16, name="mp_bf")
            nc.vector.tensor_copy(mp_bf[:], mp_sb[:])
            nc.sync.dma_start(
                ar2_in[0, 0:H].rearrange("(p t) -> p t", p=128), mp_bf[:])
            fst_bf = sb.tile([N, 2], BF16, name="fst_bf")
            nc.vector.tensor_copy(fst_bf[:], fst2[:])
            nc.scalar.dma_start(
                ar2_in[0, H:H + 2 * N].rearrange("(p s) -> p s", s=2),
                fst_bf[:])
            nc.gpsimd.collective_compute(
                "AllReduce", ALU.add,
                replica_groups=[list(range(NC))],
                ins=[ar2_in.opt()], outs=[ar2_out[:]])

            # parallel readbacks on separate queues
            ml_sb = sb.tile([128, T], BF16, name="ml_sb")
            nc.sync.dma_start(
                ml_sb[:], ar2_out[0, 0:H].rearrange("(p t) -> p t", p=128))
            fst_t = sb.tile([N, 2], BF16, name="fst_t")
            nc.scalar.dma_start(
                fst_t[:], ar2_out[0, H:H + 2 * N].rearrange("(p s) -> p s",
                                                            s=2))

            # ---------------- merge-hidden LayerNorm + tanh ---------------
            st2 = sb.tile([128, 2], F32, name="st2")
            sq_scr = sb.tile([128, T], F32, name="sq_scr")
            nc.vector.tensor_reduce(st2[:, 0:1], ml_sb[:],
                                    mybir.AxisListType.X, ALU.add)
            nc.vector.scalar_tensor_tensor(sq_scr[:], ml_sb[:], 1.0,
                                           ml_sb[:], op0=ALU.mult,
                                           op1=ALU.mult,
                                           accum_out=st2[:, 1:2])
            ps_st = ps.tile([1, 2], F32, name="ps_st", tag="pB")
            nc.tensor.matmul(ps_st[:], ones128_sb[:], st2[:],
                             start=True, stop=True)
            mr = sb.tile([1, 2], F32, name="mr")
            var = sb.tile([1, 1], F32, name="mvar")
            lnv = sb.tile([1, 1], F32, name="mlnv")
            nc.vector.tensor_scalar_mul(mr[:], ps_st[:], INV_H)
            nc.vector.scalar_tensor_tensor(var[:], mr[:, 0:1], 1.0,
                                           mr[:, 0:1], op0=ALU.mult,
                                           op1=ALU.mult)
            nc.vector.tensor_sub(var[:], mr[:, 1:2], var[:])
            nc.scalar.activation(lnv[:], var[:], AF.Ln, bias=eps1[:])
            nc.scalar.activation(mr[:, 1:2], lnv[:], AF.Exp, scale=-0.5)
            ps_bc = ps.tile([128, 2], F32, name="ps_bc", tag="pH")
            nc.tensor.matmul(ps_bc[:], onesr_sb[:], mr[:],
                             start=True, stop=True)
            mrbm = sb.tile([128, 2], F32, name="mrbm")
            nc.vector.tensor_copy(mrbm[:], ps_bc[:])
            mh_e = sb.tile([128, T], F32, name="mh_e")
            if trivial_ln:
                # exp(2*(ml - m)*r) folded into ACT scale/bias:
                # scale = 2r, bias = -2*m*r  (per-partition broadcasts)
                s2r = sb.tile([128, 1], F32, name="s2r")
                nc.vector.tensor_scalar_mul(s2r[:], mrbm[:, 1:2], 2.0)
                b2mr = sb.tile([128, 1], F32, name="b2mr")
                nc.vector.scalar_tensor_tensor(b2mr[:], s2r[:], -1.0,
                                               mrbm[:, 0:1], op0=ALU.mult,
                                               op1=ALU.mult)
                nc.scalar.activation(mh_e[:], ml_sb[:], AF.Exp,
                                     bias=b2mr[:], scale=s2r[:])
            else:
                mh_t1 = sb.tile([128, T], F32, name="mh_t1")
                nc.vector.tensor_scalar(mh_t1[:], ml_sb[:], mrbm[:, 0:1],
                                        mrbm[:, 1:2], op0=ALU.subtract,
                                        op1=ALU.mult)
                nc.vector.scalar_tensor_tensor(mh_t1[:], mh_t1[:], 1.0,
                                               gm_sb[:], op0=ALU.mult,
                                               op1=ALU.mult)
                nc.vector.scalar_tensor_tensor(mh_t1[:], mh_t1[:], 1.0,
                                               bm_sb[:], op0=ALU.mult,
                                               op1=ALU.add)
                nc.scalar.activation(mh_e[:], mh_t1[:], AF.Exp, scale=2.0)
            nc.vector.tensor_scalar_add(mh_e[:], mh_e[:], 1.0)
            mh_r = sb.tile([128, T], F32, name="mh_r")
            recipA(mh_r[:], mh_e[:])
            mh_bf = sb.tile([128, T], BF16, name="mh_bf")
            nc.vector.tensor_scalar(mh_bf[:], mh_r[:], -2.0, 1.0,
                                    op0=ALU.mult, op1=ALU.add)

            # ---------------- iou mh-half (post-AR2 PE work) --------------
            for kt in range(T, KT):
                iou_mm(kt, False, kt == KT - 1)

            # ---------------- f gate + fc = sum_n f*cells -----------------
            fmr = sb.tile([N, 2], F32, name="fmr")
            flnv = sb.tile([N, 1], F32, name="flnv")
            fvar = sb.tile([N, 1], F32, name="fvar")
            nc.vector.tensor_scalar_mul(fmr[:], fst_t[:], INV_H)
            nc.vector.scalar_tensor_tensor(fvar[:], fmr[:, 0:1], 1.0,
                                           fmr[:, 0:1], op0=ALU.mult,
                                           op1=ALU.mult)
            nc.vector.tensor_sub(fvar[:], fmr[:, 1:2], fvar[:])
            nc.scalar.activation(flnv[:], fvar[:], AF.Ln, bias=epsN[:])
            nc.scalar.activation(fmr[:, 1:2], flnv[:], AF.Exp, scale=-0.5)
            f_e = sb.tile([N, S], F32, name="f_e")
            if trivial_ln:
                # sig((x-m)*r): exp(-(x-m)*r) via scale=-r, bias=m*r
                fnr = sb.tile([N, 1], F32, name="fnr")
                nc.vector.tensor_scalar_mul(fnr[:], fmr[:, 1:2], -1.0)
                fmrb = sb.tile([N, 1], F32, name="fmrb")
                nc.vector.scalar_tensor_tensor(fmrb[:], fnr[:], -1.0,
                                               fmr[:, 0:1], op0=ALU.mult,
                                               op1=ALU.mult)
                nc.scalar.activation(f_e[:], f_lin_sb[:], AF.Exp,
                                     bias=fmrb[:], scale=fnr[:])
            else:
                ft = sb.tile([N, S], F32, name="ft")
                nc.vector.tensor_scalar(ft[:], f_lin_sb[:], fmr[:, 0:1],
                                        fmr[:, 1:2], op0=ALU.subtract,
                                        op1=ALU.mult)
                nc.vector.scalar_tensor_tensor(ft[:], ft[:], 1.0, gf_sb[:],
                                               op0=ALU.mult, op1=ALU.mult)
                nc.vector.scalar_tensor_tensor(ft[:], ft[:], 1.0, bf_sb[:],
                                               op0=ALU.mult, op1=ALU.add)
                nc.scalar.activation(f_e[:], ft[:], AF.Exp, scale=-1.0)
            nc.vector.tensor_scalar_add(f_e[:], f_e[:], 1.0)
            f_r = sb.tile([N, S], F32, name="f_r")
            recipA(f_r[:], f_e[:])
            fprod = sb.tile([N, S], F32, name="fprod")
            nc.vector.scalar_tensor_tensor(fprod[:], f_r[:], 1.0,
                                           cells_sb[:], op0=ALU.mult,
                                           op1=ALU.mult)
            ps_fc = ps.tile([1, S], F32, name="ps_fc", tag="pC")
            nc.tensor.matmul(ps_fc[:], ones32_sb[:], fprod[:],
                             start=True, stop=True)

            # ---------------- AG3: iou chunk + fc chunk -------------------
            ag3_in = dram.tile([1, 4 * S], BF16, name="ag3_in")
            ag3_out = nc.dram_tensor("ag3_out", [8, 4 * S], BF16,
                                     kind="Internal", addr_space="Shared")
            fc_bf = sb.tile([1, S], BF16, name="fc_bf")
            nc.scalar.activation(fc_bf[:], ps_fc[:], AF.Copy)
            nc.scalar.dma_start(ag3_in[0, 3 * S:4 * S].rearrange(
                "(a f) -> a f", a=1), fc_bf[:])
            ag3_stage = sb.tile([1, 3 * S], BF16, name="ag3_stage")
            nc.vector.tensor_copy(ag3_stage[:, 0:512], ps_iou[:, 0:512])
            nc.scalar.activation(ag3_stage[:, 512:3 * S],
                                 ps_iou[:, 512:3 * S], AF.Copy)
            nc.sync.dma_start(ag3_in[0, 0:3 * S].rearrange(
                "(a f) -> a f", a=1), ag3_stage[:])
            nc.gpsimd.collective_compute(
                "AllGather", ALU.bypass,
                replica_groups=[list(range(NC))],
                ins=[ag3_in.opt()], outs=[ag3_out[:]])

            # single contiguous readback: [8, 1024] chunk-major
            ag3_sb = sb.tile([8, 4 * S], BF16, name="ag3_sb")
            nc.sync.dma_start(ag3_sb[:], ag3_out[:])
            i_l = ag3_sb[:, 0 * S:1 * S]
            o_l = ag3_sb[:, 1 * S:2 * S]
            u_l = ag3_sb[:, 2 * S:3 * S]
            fc_v = ag3_sb[:, 3 * S:4 * S]

            # LayerNorm stats for i/o/u: sums in one DVE reduce, squared
            # sums on the ACT engine (Square + accumulate), in parallel
            st6 = sb.tile([8, 6], F32, name="st6")
            iou3 = ag3_sb[:, 0:3 * S].rearrange("p (g f) -> p g f", g=3)
            nc.vector.tensor_reduce(st6[:, 0:3], iou3,
                                    mybir.AxisListType.X, ALU.add)
            sqa_scr = sb.tile([8, S], F32, name="sqa_scr")
            for v, vec in enumerate((i_l, o_l, u_l)):
                nc.scalar.activation(sqa_scr[:], vec, AF.Square,
                                     accum_out=st6[:, 3 + v:4 + v])
            ps_st6 = ps.tile([1, 6], F32, name="ps_st6", tag="pA")
            nc.tensor.matmul(ps_st6[:], ones8_sb[:], st6[:],
                             start=True, stop=True)
            mr6 = sb.tile([1, 6], F32, name="mr6")
            var3 = sb.tile([1, 3], F32, name="var3")
            lnv3 = sb.tile([1, 3], F32, name="lnv3")
            nc.vector.tensor_scalar_mul(mr6[:], ps_st6[:], INV_H)
            nc.vector.scalar_tensor_tensor(var3[:], mr6[:, 0:3], 1.0,
                                           mr6[:, 0:3], op0=ALU.mult,
                                           op1=ALU.mult)
            nc.vector.tensor_sub(var3[:], mr6[:, 3:6], var3[:])
            nc.scalar.activation(lnv3[:], var3[:], AF.Ln, bias=eps1[:])
            nc.scalar.activation(mr6[:, 3:6], lnv3[:], AF.Exp, scale=-0.5)
            ps_b6 = ps.tile([8, 6], F32, name="ps_b6", tag="pB")
            nc.tensor.matmul(ps_b6[:], onesr_sb[:, 0:8], mr6[:],
                             start=True, stop=True)
            mrb6 = sb.tile([8, 6], F32, name="mrb6")
            nc.vector.tensor_copy(mrb6[:], ps_b6[:])

            if trivial_ln:
                # negated / doubled per-partition scale-bias precomputes
                nr3 = sb.tile([8, 3], F32, name="nr3")
                nc.vector.tensor_scalar_mul(nr3[:], mrb6[:, 3:6], -1.0)
                mr3 = sb.tile([8, 3], F32, name="mr3")
                nc.vector.scalar_tensor_tensor(mr3[:], mrb6[:, 0:3], 1.0,
                                               mrb6[:, 3:6], op0=ALU.mult,
                                               op1=ALU.mult)
                p2ru = sb.tile([8, 1], F32, name="p2ru")
                nc.vector.tensor_scalar_mul(p2ru[:], mrb6[:, 5:6], 2.0)
                n2mru = sb.tile([8, 1], F32, name="n2mru")
                nc.vector.tensor_scalar_mul(n2mru[:], mr3[:, 2:3], -2.0)

                e3 = sb.tile([8, 3 * S], F32, name="e3")
                nc.scalar.activation(e3[:, 0:S], i_l, AF.Exp,
                                     bias=mr3[:, 0:1], scale=nr3[:, 0:1])
                nc.scalar.activation(e3[:, S:2 * S], o_l, AF.Exp,
                                     bias=mr3[:, 1:2], scale=nr3[:, 1:2])
                nc.scalar.activation(e3[:, 2 * S:3 * S], u_l, AF.Exp,
                                     bias=n2mru[:], scale=p2ru[:])
                nc.vector.tensor_scalar_add(e3[:], e3[:], 1.0)
                r3g = sb.tile([8, 3 * S], F32, name="r3g")
                recipA(r3g[:], e3[:])
                i_g = r3g[:, 0:S]
                o_g = r3g[:, S:2 * S]
                u_r3 = r3g[:, 2 * S:3 * S]
            else:
                def gate_ln(vec, v, g_t, b_t, nm):
                    t1 = sb.tile([8, S], F32, name=nm + "_t1")
                    nc.vector.tensor_scalar(t1[:], vec,
                                            mrb6[:, v:v + 1],
                                            mrb6[:, 3 + v:4 + v],
                                            op0=ALU.subtract, op1=ALU.mult)
                    nc.vector.scalar_tensor_tensor(t1[:], t1[:], 1.0,
                                                   g_t[:], op0=ALU.mult,
                                                   op1=ALU.mult)
                    nc.vector.scalar_tensor_tensor(t1[:], t1[:], 1.0,
                                                   b_t[:], op0=ALU.mult,
                                                   op1=ALU.add)
                    return t1

                yi = gate_ln(i_l, 0, gi_sb, bi_sb, "yi")
                yo = gate_ln(o_l, 1, go_sb, bo_sb, "yo")
                yu = gate_ln(u_l, 2, gu_sb, bu_sb, "yu")

                def sig(y, nm):
                    e = sb.tile([8, S], F32, name=nm + "_e")
                    nc.scalar.activation(e[:], y[:], AF.Exp, scale=-1.0)
                    nc.vector.tensor_scalar_add(e[:], e[:], 1.0)
                    r = sb.tile([8, S], F32, name=nm + "_r")
                    recipA(r[:], e[:])
                    return r

                i_g = sig(yi, "ig")
                o_g = sig(yo, "og")
                u_e = sb.tile([8, S], F32, name="u_e")
                nc.scalar.activation(u_e[:], yu[:], AF.Exp, scale=2.0)

            if trivial_ln:
                u_rv = u_r3
                i_gv, o_gv = i_g, o_g
            else:
                nc.vector.tensor_scalar_add(u_e[:], u_e[:], 1.0)
                u_r = sb.tile([8, S], F32, name="u_r")
                recipA(u_r[:], u_e[:])
                u_rv = u_r[:]
                i_gv, o_gv = i_g[:], o_g[:]
            # cell_lin = i*u + fc = i - 2*i*ru + fc
            iru = sb.tile([8, S], F32, name="iru")
            nc.vector.scalar_tensor_tensor(iru[:], u_rv, 1.0, i_gv,
                                           op0=ALU.mult, op1=ALU.mult)
            cell_lin = sb.tile([8, S], F32, name="cell_lin")
            nc.vector.scalar_tensor_tensor(cell_lin[:], iru[:], -2.0,
                                           i_gv, op0=ALU.mult,
                                           op1=ALU.add)
            nc.vector.scalar_tensor_tensor(cell_lin[:], cell_lin[:], 1.0,
                                           fc_v, op0=ALU.mult, op1=ALU.add)

            # cell LayerNorm
            cst = sb.tile([8, 2], F32, name="cst")
            csq = sb.tile([8, S], F32, name="csq")
            nc.vector.tensor_reduce(cst[:, 0:1], cell_lin[:],
                                    mybir.AxisListType.X, ALU.add)
            nc.scalar.activation(csq[:], cell_lin[:], AF.Square,
                                 accum_out=cst[:, 1:2])
            ps_cst = ps.tile([1, 2], F32, name="ps_cst", tag="pC")
            nc.tensor.matmul(ps_cst[:], ones8_sb[:], cst[:],
                             start=True, stop=True)
            cmr = sb.tile([1, 2], F32, name="cmr")
            cvar = sb.tile([1, 1], F32, name="cvar")
            clnv = sb.tile([1, 1], F32, name="clnv")
            nc.vector.tensor_scalar_mul(cmr[:], ps_cst[:], INV_H)
            nc.vector.scalar_tensor_tensor(cvar[:], cmr[:, 0:1], 1.0,
                                           cmr[:, 0:1], op0=ALU.mult,
                                           op1=ALU.mult)
            nc.vector.tensor_sub(cvar[:], cmr[:, 1:2], cvar[:])
            nc.scalar.activation(clnv[:], cvar[:], AF.Ln, bias=eps1[:])
            nc.scalar.activation(cmr[:, 1:2], clnv[:], AF.Exp, scale=-0.5)
            ps_cb = ps.tile([8, 2], F32, name="ps_cb", tag="pD")
            nc.tensor.matmul(ps_cb[:], onesr_sb[:, 0:8], cmr[:],
                             start=True, stop=True)
            mrbc = sb.tile([8, 2], F32, name="mrbc")
            nc.vector.tensor_copy(mrbc[:], ps_cb[:])
            new_c = sb.tile([8, S], F32, name="new_c")
            t_e = sb.tile([8, S], F32, name="t_e")
            if trivial_ln:
                nc.vector.tensor_scalar(new_c[:], cell_lin[:],
                                        mrbc[:, 0:1], mrbc[:, 1:2],
                                        op0=ALU.subtract, op1=ALU.mult)
                # tanh arg: exp(2*(cell-m)*r) via scale=2r, bias=-2mr
                c2r = sb.tile([8, 1], F32, name="c2r")
                nc.vector.tensor_scalar_mul(c2r[:], mrbc[:, 1:2], 2.0)
                cn2mr = sb.tile([8, 1], F32, name="cn2mr")
                nc.vector.scalar_tensor_tensor(cn2mr[:], c2r[:], -1.0,
                                               mrbc[:, 0:1], op0=ALU.mult,
                                               op1=ALU.mult)
                nc.scalar.activation(t_e[:], cell_lin[:], AF.Exp,
                                     bias=cn2mr[:], scale=c2r[:])
            else:
                nc.vector.tensor_scalar(new_c[:], cell_lin[:],
                                        mrbc[:, 0:1], mrbc[:, 1:2],
                                        op0=ALU.subtract, op1=ALU.mult)
                nc.vector.scalar_tensor_tensor(new_c[:], new_c[:], 1.0,
                                               gc_sb[:], op0=ALU.mult,
                                               op1=ALU.mult)
                nc.vector.scalar_tensor_tensor(new_c[:], new_c[:], 1.0,
                                               bc_sb[:], op0=ALU.mult,
                                               op1=ALU.add)
                nc.scalar.activation(t_e[:], new_c[:], AF.Exp, scale=2.0)

            # new_h = o * tanh(.) = o - 2*o*rt
            nc.vector.tensor_scalar_add(t_e[:], t_e[:], 1.0)
            t_r = sb.tile([8, S], F32, name="t_r")
            recipA(t_r[:], t_e[:])
            ort = sb.tile([8, S], F32, name="ort")
            nc.vector.scalar_tensor_tensor(ort[:], t_r[:], 1.0, o_gv,
                                           op0=ALU.mult, op1=ALU.mult)
            new_h = sb.tile([8, S], F32, name="new_h")
            nc.vector.scalar_tensor_tensor(new_h[:], ort[:], -2.0, o_gv,
                                           op0=ALU.mult, op1=ALU.add)

            nc.sync.dma_start(out_c[:], new_c[:])
            nc.scalar.dma_start(out_h[:], new_h[:])

    nc.compile()
    return nc


def _tmaj(v):
    """[2048] vector -> [128,16] t-major sbuf image (sb[p,t] = v[t*128+p])."""
    return np.ascontiguousarray(v.reshape(T, 128).T)


def _ktiles(wT, cols):
    """wT: [K_in, out_cols] -> [128, (K_in/128)*cols] partition-major pack."""
    k_in = wT.shape[0]
    return np.ascontiguousarray(
        wT.reshape(k_in // 128, 128, cols).transpose(1, 0, 2).reshape(
            128, (k_in // 128) * cols))


def kernel(input, hiddens, cells, external,
           W_ai, W_attn, W_merge, W_iou, W_fi, W_fh,
           g_merge, b_merge, g_f, b_f, g_i, b_i, g_o, b_o, g_u, b_u,
           g_c, b_c):
    f32 = np.float32
    gs = [np.asarray(g, f32) for g in
          (g_merge, g_f, g_i, g_o, g_u, g_c)]
    bs = [np.asarray(b, f32) for b in
          (b_merge, b_f, b_i, b_o, b_u, b_c)]
    trivial = (all(np.all(g == 1.0) for g in gs)
               and all(np.all(b == 0.0) for b in bs))
    key = ("nc", trivial)
    if key not in _CACHE:
        _CACHE[key] = _build(trivial)
    nc = _CACHE[key]

    input = np.asarray(input, f32)
    hiddens = np.asarray(hiddens, f32)
    cells = np.asarray(cells, f32)
    external = np.asarray(external, f32)

    hTt = _ktiles(np.ascontiguousarray(hiddens.T), N).astype(NPBF)
    xT32 = _ktiles(np.tile(input[:, None], (1, N)), N).astype(NPBF)
    eT32 = _ktiles(np.tile(external[:, None], (1, N)), N).astype(NPBF)
    x1 = _tmaj(input).astype(NPBF)

    com = {
        "hT": hTt, "xT32": xT32, "eT32": eT32, "x1": x1,
        "gm": _tmaj(gs[0]), "bm": _tmaj(bs[0]),
        "gi8": gs[2].reshape(8, S), "bi8": bs[2].reshape(8, S),
        "go8": gs[3].reshape(8, S), "bo8": bs[3].reshape(8, S),
        "gu8": gs[4].reshape(8, S), "bu8": bs[4].reshape(8, S),
        "gc8": gs[5].reshape(8, S), "bc8": bs[5].reshape(8, S),
        "ones8": np.ones((8, 1), f32), "ones32": np.ones((N, 1), f32),
        "ones128": np.ones((128, 1), f32),
        "onesr": np.ones((1, 128), f32),
    }

    Wf_cat = np.concatenate([W_fh, W_fi], axis=1)              # [H, 4096]
    in_maps = []
    for c in range(NC):
        r = slice(c * S, (c + 1) * S)
        iou_rows = np.concatenate(
            [W_iou[g * H + c * S:g * H + (c + 1) * S, :] for g in range(3)],
            axis=0)                                            # [768, 4096]
        m = dict(com)
        m.update({
            "hTc": np.ascontiguousarray(
                hiddens.T[c * S:(c + 1) * S].reshape(2, 128, N)
                .transpose(1, 0, 2).reshape(128, 2 * N)).astype(NPBF),
            "cells_chunk": np.ascontiguousarray(cells[:, r]),
            "gf_rep": np.tile(gs[1][r], (N, 1)),
            "bf_rep": np.tile(bs[1][r], (N, 1)),
            "wattn_rep": np.tile(np.asarray(W_attn, f32)[0, r], (N, 1)),
            "watsum": np.full((N, 1), np.asarray(W_attn, f32)[0, r].sum(),
                              f32),
            "wf": _ktiles(np.ascontiguousarray(Wf_cat[r].T), S).astype(NPBF),
            "wai": _ktiles(np.ascontiguousarray(W_ai[r].T), S).astype(NPBF),
            "wmg": _ktiles(np.ascontiguousarray(W_merge[:, r].T),
                           H).astype(NPBF),
            "wiou": _ktiles(np.ascontiguousarray(iou_rows.T),
                            3 * S).astype(NPBF),
        })
        in_maps.append({k: (np.ascontiguousarray(v) if v.dtype == NPBF
                            else np.ascontiguousarray(v, f32))
                        for k, v in m.items()})

    res = run_bass_kernel_spmd(nc, in_maps, core_ids=list(range(NC)))
    _CACHE["last_results"] = res
    r0 = res.results[0]
    new_h = r0["out_h"].reshape(H).astype(f32)
    new_c = r0["out_c"].reshape(H).astype(f32)
    return new_h, new_c


# revision 14
# speedup vs baseline: 1.1609x; 1.0916x over previous
"""AttentiveChildSumTreeLSTMCell on 8 Trainium2 NeuronCores.

Tensor-parallel: column-parallel f/attention/iou linears (hidden dim sharded
8 ways), row-parallel merge linear.  Collectives: two zero-dependency warmup
AllGathers (absorb communicator init + first-collective software setup),
AllGather of partial attention logits, AllReduce of merge-linear partials +
f LayerNorm stats, AllGather of iou/forget*cell chunks.  Matmul operands
are bf16; accumulation and all norm/gate math stays fp32.

All activations use a single ACT table set (ln+exp, loaded once): sigmoid
and tanh are computed via exp + DVE fast-reciprocal, LayerNorm rstd via
exp(-0.5*ln(var+eps)).  When all LayerNorm gains are 1 and biases 0 (the
common case, verified at runtime), the (x-mean)*rstd normalization is
folded into the exp activation's per-partition scale/bias operands.  The
gpsimd queue carries only collective triggers; the final gate math runs in
a [8, 256] chunk layout read straight from the AllGather result.
"""

import sys

for _p in ("/opt/trn_rl_repo",):
    if _p not in sys.path:
        sys.path.insert(0, _p)

import ml_dtypes
import numpy as np

import concourse.bacc as bacc
import concourse.mybir as mybir
import concourse.tile as tile
from concourse.bass_utils import run_bass_kernel_spmd
from concourse.tile_rust import add_dep_helper

F32 = mybir.dt.float32
BF16 = mybir.dt.bfloat16
AF = mybir.ActivationFunctionType
ALU = mybir.AluOpType
NPBF = ml_dtypes.bfloat16

H = 2048
N = 32
NC = 8
S = H // NC           # 256: per-core chunk of every sharded dim
T = H // 128          # 16 tiles of 128 along a 2048 dim
KT = 32               # K-tiles along the 4096 contraction dims
EPS = 1e-5
INV_H = 1.0 / H

# index of the ln+exp activation-function set in act_info.json
LN_EXP_SET = 6

_CACHE = {}


def _build(trivial_ln):
    nc = bacc.Bacc(None, target_bir_lowering=False, debug=False, num_devices=NC)

    def din(name, shape, dt=F32):
        return nc.dram_tensor(name, list(shape), dt, kind="ExternalInput")

    # ---- per-core DRAM inputs (SPMD: same shapes on every core) ----
    hT = din("hT", (128, T * N), BF16)
    xT32 = din("xT32", (128, T * N), BF16)
    eT32 = din("eT32", (128, T * N), BF16)
    x1 = din("x1", (128, T), BF16)
    hTc = din("hTc", (128, 2 * N), BF16)
    cells_chunk = din("cells_chunk", (N, S))
    gf_rep = din("gf_rep", (N, S))
    bf_rep = din("bf_rep", (N, S))
    wattn_rep = din("wattn_rep", (N, S))
    watsum = din("watsum", (N, 1))
    gm = din("gm", (128, T))
    bm = din("bm", (128, T))
    gi8 = din("gi8", (8, S))
    bi8 = din("bi8", (8, S))
    go8 = din("go8", (8, S))
    bo8 = din("bo8", (8, S))
    gu8 = din("gu8", (8, S))
    bu8 = din("bu8", (8, S))
    gc8 = din("gc8", (8, S))
    bc8 = din("bc8", (8, S))
    ones8 = din("ones8", (8, 1))
    ones32 = din("ones32", (N, 1))
    ones128 = din("ones128", (128, 1))
    onesr = din("onesr", (1, 128))
    wf = din("wf", (128, KT * S), BF16)        # [W_fh | W_fi]^T chunk
    wai = din("wai", (128, KT * S), BF16)      # W_ai^T chunk
    wmg = din("wmg", (128, 2 * H), BF16)       # W_merge^T in-chunk
    wiou = din("wiou", (128, KT * 3 * S), BF16)  # W_iou^T chunk (i|o|u cols)

    out_h = nc.dram_tensor("out_h", [8, S], F32, kind="ExternalOutput")
    out_c = nc.dram_tensor("out_c", [8, S], F32, kind="ExternalOutput")

    with tile.TileContext(nc) as tc:
        with (
            tc.tile_pool(name="sb", bufs=1) as sb,
            tc.tile_pool(name="ps", bufs=1, space="PSUM") as ps,
            tc.tile_pool(name="dram", bufs=1, space="DRAM") as dram,
        ):
            # ---- warmup collectives: zero deps, trigger at t~0 -----------
            # Contents are irrelevant; they pull communicator init and
            # per-kind first-collective software setup off the critical path.
            warm_in = dram.tile([1, 16], F32, name="warm_in")
            warm_out = dram.tile([8, 16], F32, name="warm_out")
            with tc.high_priority():
                nc.gpsimd.collective_compute(
                    "AllGather", ALU.bypass,
                    replica_groups=[list(range(NC))],
                    ins=[warm_in.opt()], outs=[warm_out.opt()])
                # single ACT table load for the whole kernel (ln+exp set)
                nc.scalar.add_instruction(mybir.InstLoadActFuncSet(
                    name=f"I-{nc.next_id()}", ins=[], outs=[],
                    act_func_set_id=LN_EXP_SET))

            # ------- small resident loads (scalar queue; SP is weights) ---
            def load(t_dram, shape, dt=F32):
                t_sb = sb.tile(shape, dt, name=t_dram.name + "_sb")
                nc.scalar.dma_start(t_sb[:], t_dram[:])
                return t_sb

            hT_sb = load(hT, [128, T, N], BF16)
            xT32_sb = load(xT32, [128, T, N], BF16)
            eT32_sb = load(eT32, [128, T, N], BF16)
            x1_sb = load(x1, [128, T], BF16)
            hTc_sb = load(hTc, [128, 2, N], BF16)
            cells_sb = load(cells_chunk, [N, S])
            wat_sb = load(wattn_rep, [N, S])
            wsum_sb = load(watsum, [N, 1])
            ones8_sb = load(ones8, [8, 1])
            ones32_sb = load(ones32, [N, 1])
            ones128_sb = load(ones128, [128, 1])
            onesr_sb = load(onesr, [1, 128])
            if not trivial_ln:
                gf_sb = load(gf_rep, [N, S])
                bf_sb = load(bf_rep, [N, S])
                gm_sb = load(gm, [128, T])
                bm_sb = load(bm, [128, T])
                gi_sb = load(gi8, [8, S])
                bi_sb = load(bi8, [8, S])
                go_sb = load(go8, [8, S])
                bo_sb = load(bo8, [8, S])
                gu_sb = load(gu8, [8, S])
                bu_sb = load(bu8, [8, S])
                gc_sb = load(gc8, [8, S])
                bc_sb = load(bc8, [8, S])

            eps1 = sb.tile([1, 1], F32, name="eps1")
            nc.vector.memset(eps1[:], EPS)
            epsN = sb.tile([N, 1], F32, name="epsN")
            nc.vector.memset(epsN[:], EPS)

            def recipA(out, in_):
                nc.vector.reciprocal_approx_fast(out=out, in_=in_)

            # ---------------- weight streaming DMAs (SP, ordered) ---------
            wai_sb = sb.tile([128, KT * S], BF16, name="wai_sb")
            wf_sb = sb.tile([128, KT * S], BF16, name="wf_sb")
            wiou_sb = sb.tile([128, KT * 3 * S], BF16, name="wiou_sb")
            wmg_sb = sb.tile([128, 2 * H], BF16, name="wmg_sb")

            wdmas = []
            for k in range(2):  # wai: 2 x 1MB
                wdmas.append(nc.sync.dma_start(
                    wai_sb[:, k * 4096:(k + 1) * 4096],
                    wai[:, k * 4096:(k + 1) * 4096]))
            for k in range(2):  # wf: 2 x 1MB
                wdmas.append(nc.sync.dma_start(
                    wf_sb[:, k * 4096:(k + 1) * 4096],
                    wf[:, k * 4096:(k + 1) * 4096]))
            for k in range(3):  # wiou x-half: 3 x 1MB
                wdmas.append(nc.sync.dma_start(
                    wiou_sb[:, k * 4096:(k + 1) * 4096],
                    wiou[:, k * 4096:(k + 1) * 4096]))
            wdmas.append(nc.sync.dma_start(wmg_sb[:], wmg[:]))  # 1MB
            for k in range(3, 6):  # wiou mh-half: 3 x 1MB
                wdmas.append(nc.sync.dma_start(
                    wiou_sb[:, k * 4096:(k + 1) * 4096],
                    wiou[:, k * 4096:(k + 1) * 4096]))
            # chain three-deep: keeps arrival order without strangling BW
            for i in range(3, len(wdmas)):
                add_dep_helper(wdmas[i].ins, wdmas[i - 3].ins, sync=True,
                               reason="weight DMA arrival order")

            # ---------------- attention: ai -> partial logits -------------
            ps_ai = ps.tile([N, S], F32, name="ps_ai", tag="pA")
            for kt in range(KT):
                act = hT_sb if kt < T else eT32_sb
                nc.tensor.matmul(ps_ai[:], act[:, kt % T, :],
                                 wai_sb[:, kt * S:(kt + 1) * S],
                                 start=(kt == 0), stop=(kt == KT - 1))
            # logits = sum_j wat * tanh(ai); tanh(w) = 2/(1+exp(-2w)) - 1
            # lg = 2 * sum(wat * r) - sum(wat),  r = 1/(1+exp(-2w))
            ai_e = sb.tile([N, S], F32, name="ai_e")
            nc.scalar.activation(ai_e[:], ps_ai[:], AF.Exp, scale=-2.0)
            ai_d = sb.tile([N, S], F32, name="ai_d")
            nc.vector.tensor_scalar_add(ai_d[:], ai_e[:], 1.0)
            ai_r = sb.tile([N, S], F32, name="ai_r")
            recipA(ai_r[:], ai_d[:])
            aw = sb.tile([N, S], F32, name="aw")
            asum = sb.tile([N, 1], F32, name="asum")
            nc.vector.scalar_tensor_tensor(aw[:], ai_r[:], 1.0, wat_sb[:],
                                           op0=ALU.mult, op1=ALU.mult,
                                           accum_out=asum[:])
            lg_sb = sb.tile([N, 1], F32, name="lg_sb")
            nc.vector.tensor_scalar(lg_sb[:], asum[:], 2.0, wsum_sb[:],
                                    op0=ALU.mult, op1=ALU.subtract)

            # ---------------- AG1: partial logits -------------------------
            ag1_in = dram.tile([1, N], F32, name="ag1_in")
            ag1_out_t = nc.dram_tensor("ag1_out", [8, N], F32,
                                       kind="Internal", addr_space="Shared")
            nc.scalar.dma_start(
                ag1_in[0, :].rearrange("(p one) -> p one", one=1), lg_sb[:])
            nc.gpsimd.collective_compute(
                "AllGather", ALU.bypass,
                replica_groups=[list(range(NC))],
                ins=[ag1_in.opt()], outs=[ag1_out_t[:]])

            # ---------------- f_lin + per-child stats (speculative) -------
            ps_f = ps.tile([N, S], F32, name="ps_f", tag="pG")
            for kt in range(KT):
                act = hT_sb if kt < T else xT32_sb
                nc.tensor.matmul(ps_f[:], act[:, kt % T, :],
                                 wf_sb[:, kt * S:(kt + 1) * S],
                                 start=(kt == 0), stop=(kt == KT - 1))
            f_lin_sb = sb.tile([N, S], F32, name="f_lin_sb")
            fst2 = sb.tile([N, 2], F32, name="fst2")
            fsq_scr = sb.tile([N, S], F32, name="fsq_scr")
            nc.vector.tensor_copy(f_lin_sb[:], ps_f[:])
            nc.vector.tensor_reduce(fst2[:, 0:1], f_lin_sb[:],
                                    mybir.AxisListType.X, ALU.add)
            nc.vector.scalar_tensor_tensor(fsq_scr[:], f_lin_sb[:], 1.0,
                                           f_lin_sb[:], op0=ALU.mult,
                                           op1=ALU.mult,
                                           accum_out=fst2[:, 1:2])

            # speculative per-child merge projections (pre-attention):
            # M[p, t, n] = sum_in W_merge[t*128+p, in] * h[n, in], in-chunk
            ps_M = ps.tile([128, T, N], F32, name="ps_M", tag="pD")
            for t in range(T):
                for s in range(2):
                    nc.tensor.matmul(
                        ps_M[:, t, :],
                        wmg_sb[:, s * H + t * 128: s * H + (t + 1) * 128],
                        hTc_sb[:, s, :],
                        start=(s == 0), stop=(s == 1))

            # ---------------- iou x-half (speculative) --------------------
            ps_iou = ps.tile([1, 3 * S], F32, name="ps_iou", tag="pIOU")
            nslices = ((0, 512), (512, 768))

            def iou_mm(kt, start, stop):
                lhs = (x1_sb[:, kt:kt + 1] if kt < T
                       else mh_bf[:, kt - T:kt - T + 1])
                for c0, c1 in nslices:
                    nc.tensor.matmul(ps_iou[:, c0:c1],
                                     lhs, wiou_sb[:, kt * 768 + c0:
                                                  kt * 768 + c1],
                                     start=start, stop=stop)

            for kt in range(T):          # x half: runs as weights land
                iou_mm(kt, kt == 0, False)

            # ---------------- post-AG1: softmax + merge partials ----------
            ag1_sb = sb.tile([8, N], F32, name="ag1_sb")
            nc.sync.dma_start(ag1_sb[:], ag1_out_t[:])
            ps_l2r = ps.tile([1, N], F32, name="ps_l2r", tag="pB")
            nc.tensor.matmul(ps_l2r[:], ones8_sb[:], ag1_sb[:],
                             start=True, stop=True)
            exps_row = sb.tile([1, N], F32, name="exps_row")
            # softmax without max-subtraction or normalization: the scale
            # cancels inside the merge LayerNorm
            nc.scalar.activation(exps_row[:], ps_l2r[:], AF.Exp)
            ps_eb = ps.tile([128, N], F32, name="ps_eb", tag="pH")
            nc.tensor.matmul(ps_eb[:], onesr_sb[:], exps_row[:],
                             start=True, stop=True)

            # merge-linear partials: one multiply + one reduce
            exps_b = sb.tile([128, N], F32, name="exps_b")
            nc.vector.tensor_copy(exps_b[:], ps_eb[:])
            mp_sb = sb.tile([128, T], F32, name="mp_sb")
            mp_scr3 = sb.tile([128, T, N], F32, name="mp_scr3")
            eb3 = exps_b[:].rearrange("p (one n) -> p one n",
                                      one=1).to_broadcast((128, T, N))
            nc.vector.scalar_tensor_tensor(mp_scr3[:], ps_M[:], 1.0, eb3,
                                           op0=ALU.mult, op1=ALU.mult)
            nc.vector.tensor_reduce(mp_sb[:], mp_scr3[:],
                                    mybir.AxisListType.X, ALU.add)

            # PE p-state warmers: junk matmuls on the in-order PE queue
            # right after the post-AG1 matmuls, so the tensor engine stays
            # at high clock through the AR2 wait and the iou mh-half block
            # runs at full speed.  Results are never read.
            ps_warm = ps.tile([1, 512], F32, name="ps_warm", tag="pB")
            for w in range(14):
                nc.tensor.matmul(ps_warm[:], x1_sb[:, 0:1],
                                 wiou_sb[:, 0:512],
                                 start=True, stop=True)

            # ---------------- AR2: merge partials + f stats ---------------
            ar2_in = dram.tile([1, H + 2 * N], BF16, name="ar2_in")
            ar2_out = nc.dram_tensor("ar2_out", [1, H + 2 * N], BF16,
                                     kind="Internal", addr_space="Shared")
            mp_bf = sb.tile([128, T], BF16, name="mp_bf")
            nc.vector.tensor_copy(mp_bf[:], mp_sb[:])
            nc.sync.dma_start(
                ar2_in[0, 0:H].rearrange("(p t) -> p t", p=128), mp_bf[:])
            fst_bf = sb.tile([N, 2], BF16, name="fst_bf")
            nc.vector.tensor_copy(fst_bf[:], fst2[:])
            nc.scalar.dma_start(
                ar2_in[0, H:H + 2 * N].rearrange("(p s) -> p s", s=2),
                fst_bf[:])
            nc.gpsimd.collective_compute(
                "AllReduce", ALU.add,
                replica_groups=[list(range(NC))],
                ins=[ar2_in.opt()], outs=[ar2_out[:]])

            # parallel readbacks on separate queues
            ml_sb = sb.tile([128, T], BF16, name="ml_sb")
            nc.sync.dma_start(
                ml_sb[:], ar2_out[0, 0:H].rearrange("(p t) -> p t", p=128))
            fst_t = sb.tile([N, 2], BF16, name="fst_t")
            nc.scalar.dma_start(
                fst_t[:], ar2_out[0, H:H + 2 * N].rearrange("(p s) -> p s",
                                                            s=2))

            # ---------------- merge-hidden LayerNorm + tanh ---------------
            st2 = sb.tile([128, 2], F32, name="st2")
            sq_scr = sb.tile([128, T], F32, name="sq_scr")
            nc.vector.tensor_reduce(st2[:, 0:1], ml_sb[:],
                                    mybir.AxisListType.X, ALU.add)
            nc.vector.scalar_tensor_tensor(sq_scr[:], ml_sb[:], 1.0,
                                           ml_sb[:], op0=ALU.mult,
                                           op1=ALU.mult,
                                           accum_out=st2[:, 1:2])
            ps_st = ps.tile([1, 2], F32, name="ps_st", tag="pB")
            nc.tensor.matmul(ps_st[:], ones128_sb[:], st2[:],
                             start=True, stop=True)
            mr = sb.tile([1, 2], F32, name="mr")
            var = sb.tile([1, 1], F32, name="mvar")
            lnv = sb.tile([1, 1], F32, name="mlnv")
            nc.vector.tensor_scalar_mul(mr[:], ps_st[:], INV_H)
            nc.vector.scalar_tensor_tensor(var[:], mr[:, 0:1], 1.0,
                                           mr[:, 0:1], op0=ALU.mult,
                                           op1=ALU.mult)
            nc.vector.tensor_sub(var[:], mr[:, 1:2], var[:])
            nc.scalar.activation(lnv[:], var[:], AF.Ln, bias=eps1[:])
            nc.scalar.activation(mr[:, 1:2], lnv[:], AF.Exp, scale=-0.5)
            ps_bc = ps.tile([128, 2], F32, name="ps_bc", tag="pH")
            nc.tensor.matmul(ps_bc[:], onesr_sb[:], mr[:],
                             start=True, stop=True)
            mrbm = sb.tile([128, 2], F32, name="mrbm")
            nc.vector.tensor_copy(mrbm[:], ps_bc[:])
            mh_e = sb.tile([128, T], F32, name="mh_e")
            if trivial_ln:
                # exp(2*(ml - m)*r) folded into ACT scale/bias:
                # scale = 2r, bias = -2*m*r  (per-partition broadcasts)
                s2r = sb.tile([128, 1], F32, name="s2r")
                nc.vector.tensor_scalar_mul(s2r[:], mrbm[:, 1:2], 2.0)
                b2mr = sb.tile([128, 1], F32, name="b2mr")
                nc.vector.scalar_tensor_tensor(b2mr[:], s2r[:], -1.0,
                                               mrbm[:, 0:1], op0=ALU.mult,
                                               op1=ALU.mult)
                nc.scalar.activation(mh_e[:], ml_sb[:], AF.Exp,
                                     bias=b2mr[:], scale=s2r[:])
            else:
                mh_t1 = sb.tile([128, T], F32, name="mh_t1")
                nc.vector.tensor_scalar(mh_t1[:], ml_sb[:], mrbm[:, 0:1],
                                        mrbm[:, 1:2], op0=ALU.subtract,
                                        op1=ALU.mult)
                nc.vector.scalar_tensor_tensor(mh_t1[:], mh_t1[:], 1.0,
                                               gm_sb[:], op0=ALU.mult,
                                               op1=ALU.mult)
                nc.vector.scalar_tensor_tensor(mh_t1[:], mh_t1[:], 1.0,
                                               bm_sb[:], op0=ALU.mult,
                                               op1=ALU.add)
                nc.scalar.activation(mh_e[:], mh_t1[:], AF.Exp, scale=2.0)
            nc.vector.tensor_scalar_add(mh_e[:], mh_e[:], 1.0)
            mh_r = sb.tile([128, T], F32, name="mh_r")
            recipA(mh_r[:], mh_e[:])
            mh_bf = sb.tile([128, T], BF16, name="mh_bf")
            nc.vector.tensor_scalar(mh_bf[:], mh_r[:], -2.0, 1.0,
                                    op0=ALU.mult, op1=ALU.add)

            # ---------------- iou mh-half (post-AR2 PE work) --------------
            for kt in range(T, KT):
                iou_mm(kt, False, kt == KT - 1)

            # ---------------- f gate + fc = sum_n f*cells -----------------
            fmr = sb.tile([N, 2], F32, name="fmr")
            flnv = sb.tile([N, 1], F32, name="flnv")
            fvar = sb.tile([N, 1], F32, name="fvar")
            nc.vector.tensor_scalar_mul(fmr[:], fst_t[:], INV_H)
            nc.vector.scalar_tensor_tensor(fvar[:], fmr[:, 0:1], 1.0,
                                           fmr[:, 0:1], op0=ALU.mult,
                                           op1=ALU.mult)
            nc.vector.tensor_sub(fvar[:], fmr[:, 1:2], fvar[:])
            nc.scalar.activation(flnv[:], fvar[:], AF.Ln, bias=epsN[:])
            nc.scalar.activation(fmr[:, 1:2], flnv[:], AF.Exp, scale=-0.5)
            f_e = sb.tile([N, S], F32, name="f_e")
            if trivial_ln:
                # sig((x-m)*r): exp(-(x-m)*r) via scale=-r, bias=m*r
                fnr = sb.tile([N, 1], F32, name="fnr")
                nc.vector.tensor_scalar_mul(fnr[:], fmr[:, 1:2], -1.0)
                fmrb = sb.tile([N, 1], F32, name="fmrb")
                nc.vector.scalar_tensor_tensor(fmrb[:], fnr[:], -1.0,
                                               fmr[:, 0:1], op0=ALU.mult,
                                               op1=ALU.mult)
                nc.scalar.activation(f_e[:], f_lin_sb[:], AF.Exp,
                                     bias=fmrb[:], scale=fnr[:])
            else:
                ft = sb.tile([N, S], F32, name="ft")
                nc.vector.tensor_scalar(ft[:], f_lin_sb[:], fmr[:, 0:1],
                                        fmr[:, 1:2], op0=ALU.subtract,
                                        op1=ALU.mult)
                nc.vector.scalar_tensor_tensor(ft[:], ft[:], 1.0, gf_sb[:],
                                               op0=ALU.mult, op1=ALU.mult)
                nc.vector.scalar_tensor_tensor(ft[:], ft[:], 1.0, bf_sb[:],
                                               op0=ALU.mult, op1=ALU.add)
                nc.scalar.activation(f_e[:], ft[:], AF.Exp, scale=-1.0)
            nc.vector.tensor_scalar_add(f_e[:], f_e[:], 1.0)
            f_r = sb.tile([N, S], F32, name="f_r")
            recipA(f_r[:], f_e[:])
            fprod = sb.tile([N, S], F32, name="fprod")
            nc.vector.scalar_tensor_tensor(fprod[:], f_r[:], 1.0,
                                           cells_sb[:], op0=ALU.mult,
                                           op1=ALU.mult)
            ps_fc = ps.tile([1, S], F32, name="ps_fc", tag="pC")
            nc.tensor.matmul(ps_fc[:], ones32_sb[:], fprod[:],
                             start=True, stop=True)

            # ---------------- AG3: iou chunk + fc chunk -------------------
            ag3_in = dram.tile([1, 4 * S], BF16, name="ag3_in")
            ag3_out = nc.dram_tensor("ag3_out", [8, 4 * S], BF16,
                                     kind="Internal", addr_space="Shared")
            fc_bf = sb.tile([1, S], BF16, name="fc_bf")
            nc.scalar.activation(fc_bf[:], ps_fc[:], AF.Copy)
            nc.scalar.dma_start(ag3_in[0, 3 * S:4 * S].rearrange(
                "(a f) -> a f", a=1), fc_bf[:])
            ag3_stage = sb.tile([1, 3 * S], BF16, name="ag3_stage")
            nc.vector.tensor_copy(ag3_stage[:, 0:512], ps_iou[:, 0:512])
            nc.scalar.activation(ag3_stage[:, 512:3 * S],
                                 ps_iou[:, 512:3 * S], AF.Copy)
            nc.sync.dma_start(ag3_in[0, 0:3 * S].rearrange(
                "(a f) -> a f", a=1), ag3_stage[:])
            nc.gpsimd.collective_compute(
                "AllGather", ALU.bypass,
                replica_groups=[list(range(NC))],
                ins=[ag3_in.opt()], outs=[ag3_out[:]])

            ps_warm2 = ps.tile([1, 512], F32, name="ps_warm2", tag="pH")
            for w in range(8):
                nc.tensor.matmul(ps_warm2[:], x1_sb[:, 0:1],
                                 wiou_sb[:, 0:512],
                                 start=True, stop=True)

            # single contiguous readback: [8, 1024] chunk-major
            ag3_sb = sb.tile([8, 4 * S], BF16, name="ag3_sb")
            nc.sync.dma_start(ag3_sb[:], ag3_out[:])
            i_l = ag3_sb[:, 0 * S:1 * S]
            o_l = ag3_sb[:, 1 * S:2 * S]
            u_l = ag3_sb[:, 2 * S:3 * S]
            fc_v = ag3_sb[:, 3 * S:4 * S]

            # LayerNorm stats for i/o/u: sums in one DVE reduce, squared
            # sums on the ACT engine (Square + accumulate), in parallel
            st6 = sb.tile([8, 6], F32, name="st6")
            iou3 = ag3_sb[:, 0:3 * S].rearrange("p (g f) -> p g f", g=3)
            nc.vector.tensor_reduce(st6[:, 0:3], iou3,
                                    mybir.AxisListType.X, ALU.add)
            sqa_scr = sb.tile([8, S], F32, name="sqa_scr")
            for v, vec in enumerate((i_l, o_l, u_l)):
                nc.scalar.activation(sqa_scr[:], vec, AF.Square,
                                     accum_out=st6[:, 3 + v:4 + v])
            ps_st6 = ps.tile([1, 6], F32, name="ps_st6", tag="pA")
            nc.tensor.matmul(ps_st6[:], ones8_sb[:], st6[:],
                             start=True, stop=True)
            mr6 = sb.tile([1, 6], F32, name="mr6")
            var3 = sb.tile([1, 3], F32, name="var3")
            lnv3 = sb.tile([1, 3], F32, name="lnv3")
            nc.vector.tensor_scalar_mul(mr6[:], ps_st6[:], INV_H)
            nc.vector.scalar_tensor_tensor(var3[:], mr6[:, 0:3], 1.0,
                                           mr6[:, 0:3], op0=ALU.mult,
                                           op1=ALU.mult)
            nc.vector.tensor_sub(var3[:], mr6[:, 3:6], var3[:])
            nc.scalar.activation(lnv3[:], var3[:], AF.Ln, bias=eps1[:])
            nc.scalar.activation(mr6[:, 3:6], lnv3[:], AF.Exp, scale=-0.5)
            ps_b6 = ps.tile([8, 6], F32, name="ps_b6", tag="pB")
            nc.tensor.matmul(ps_b6[:], onesr_sb[:, 0:8], mr6[:],
                             start=True, stop=True)
            mrb6 = sb.tile([8, 6], F32, name="mrb6")
            nc.vector.tensor_copy(mrb6[:], ps_b6[:])

            if trivial_ln:
                # negated / doubled per-partition scale-bias precomputes
                nr3 = sb.tile([8, 3], F32, name="nr3")
                nc.vector.tensor_scalar_mul(nr3[:], mrb6[:, 3:6], -1.0)
                mr3 = sb.tile([8, 3], F32, name="mr3")
                nc.vector.scalar_tensor_tensor(mr3[:], mrb6[:, 0:3], 1.0,
                                               mrb6[:, 3:6], op0=ALU.mult,
                                               op1=ALU.mult)
                p2ru = sb.tile([8, 1], F32, name="p2ru")
                nc.vector.tensor_scalar_mul(p2ru[:], mrb6[:, 5:6], 2.0)
                n2mru = sb.tile([8, 1], F32, name="n2mru")
                nc.vector.tensor_scalar_mul(n2mru[:], mr3[:, 2:3], -2.0)

                e3 = sb.tile([8, 3 * S], F32, name="e3")
                nc.scalar.activation(e3[:, 0:S], i_l, AF.Exp,
                                     bias=mr3[:, 0:1], scale=nr3[:, 0:1])
                nc.scalar.activation(e3[:, S:2 * S], o_l, AF.Exp,
                                     bias=mr3[:, 1:2], scale=nr3[:, 1:2])
                nc.scalar.activation(e3[:, 2 * S:3 * S], u_l, AF.Exp,
                                     bias=n2mru[:], scale=p2ru[:])
                nc.vector.tensor_scalar_add(e3[:], e3[:], 1.0)
                r3g = sb.tile([8, 3 * S], F32, name="r3g")
                recipA(r3g[:], e3[:])
                i_g = r3g[:, 0:S]
                o_g = r3g[:, S:2 * S]
                u_r3 = r3g[:, 2 * S:3 * S]
            else:
                def gate_ln(vec, v, g_t, b_t, nm):
                    t1 = sb.tile([8, S], F32, name=nm + "_t1")
                    nc.vector.tensor_scalar(t1[:], vec,
                                            mrb6[:, v:v + 1],
                                            mrb6[:, 3 + v:4 + v],
                                            op0=ALU.subtract, op1=ALU.mult)
                    nc.vector.scalar_tensor_tensor(t1[:], t1[:], 1.0,
                                                   g_t[:], op0=ALU.mult,
                                                   op1=ALU.mult)
                    nc.vector.scalar_tensor_tensor(t1[:], t1[:], 1.0,
                                                   b_t[:], op0=ALU.mult,
                                                   op1=ALU.add)
                    return t1

                yi = gate_ln(i_l, 0, gi_sb, bi_sb, "yi")
                yo = gate_ln(o_l, 1, go_sb, bo_sb, "yo")
                yu = gate_ln(u_l, 2, gu_sb, bu_sb, "yu")

                def sig(y, nm):
                    e = sb.tile([8, S], F32, name=nm + "_e")
                    nc.scalar.activation(e[:], y[:], AF.Exp, scale=-1.0)
                    nc.vector.tensor_scalar_add(e[:], e[:], 1.0)
                    r = sb.tile([8, S], F32, name=nm + "_r")
                    recipA(r[:], e[:])
                    return r

                i_g = sig(yi, "ig")
                o_g = sig(yo, "og")
                u_e = sb.tile([8, S], F32, name="u_e")
                nc.scalar.activation(u_e[:], yu[:], AF.Exp, scale=2.0)

            if trivial_ln:
                u_rv = u_r3
                i_gv, o_gv = i_g, o_g
            else:
                nc.vector.tensor_scalar_add(u_e[:], u_e[:], 1.0)
                u_r = sb.tile([8, S], F32, name="u_r")
                recipA(u_r[:], u_e[:])
                u_rv = u_r[:]
                i_gv, o_gv = i_g[:], o_g[:]
            # cell_lin = i*u + fc = i - 2*i*ru + fc
            iru = sb.tile([8, S], F32, name="iru")
            nc.vector.scalar_tensor_tensor(iru[:], u_rv, 1.0, i_gv,
                                           op0=ALU.mult, op1=ALU.mult)
            cell_lin = sb.tile([8, S], F32, name="cell_lin")
            nc.vector.scalar_tensor_tensor(cell_lin[:], iru[:], -2.0,
                                           i_gv, op0=ALU.mult,
                                           op1=ALU.add)
            nc.vector.scalar_tensor_tensor(cell_lin[:], cell_lin[:], 1.0,
                                           fc_v, op0=ALU.mult, op1=ALU.add)

            # cell LayerNorm
            cst = sb.tile([8, 2], F32, name="cst")
            csq = sb.tile([8, S], F32, name="csq")
            nc.vector.tensor_reduce(cst[:, 0:1], cell_lin[:],
                                    mybir.AxisListType.X, ALU.add)
            nc.scalar.activation(csq[:], cell_lin[:], AF.Square,
                                 accum_out=cst[:, 1:2])
            ps_cst = ps.tile([1, 2], F32, name="ps_cst", tag="pC")
            nc.tensor.matmul(ps_cst[:], ones8_sb[:], cst[:],
                             start=True, stop=True)
            cmr = sb.tile([1, 2], F32, name="cmr")
            cvar = sb.tile([1, 1], F32, name="cvar")
            clnv = sb.tile([1, 1], F32, name="clnv")
            nc.vector.tensor_scalar_mul(cmr[:], ps_cst[:], INV_H)
            nc.vector.scalar_tensor_tensor(cvar[:], cmr[:, 0:1], 1.0,
                                           cmr[:, 0:1], op0=ALU.mult,
                                           op1=ALU.mult)
            nc.vector.tensor_sub(cvar[:], cmr[:, 1:2], cvar[:])
            nc.scalar.activation(clnv[:], cvar[:], AF.Ln, bias=eps1[:])
            nc.scalar.activation(cmr[:, 1:2], clnv[:], AF.Exp, scale=-0.5)
            ps_cb = ps.tile([8, 2], F32, name="ps_cb", tag="pD")
            nc.tensor.matmul(ps_cb[:], onesr_sb[:, 0:8], cmr[:],
                             start=True, stop=True)
            mrbc = sb.tile([8, 2], F32, name="mrbc")
            nc.vector.tensor_copy(mrbc[:], ps_cb[:])
            new_c = sb.tile([8, S], F32, name="new_c")
            t_e = sb.tile([8, S], F32, name="t_e")
            if trivial_ln:
                nc.vector.tensor_scalar(new_c[:], cell_lin[:],
                                        mrbc[:, 0:1], mrbc[:, 1:2],
                                        op0=ALU.subtract, op1=ALU.mult)
                # tanh arg: exp(2*(cell-m)*r) via scale=2r, bias=-2mr
                c2r = sb.tile([8, 1], F32, name="c2r")
                nc.vector.tensor_scalar_mul(c2r[:], mrbc[:, 1:2], 2.0)
                cn2mr = sb.tile([8, 1], F32, name="cn2mr")
                nc.vector.scalar_tensor_tensor(cn2mr[:], c2r[:], -1.0,
                                               mrbc[:, 0:1], op0=ALU.mult,
                                               op1=ALU.mult)
                nc.scalar.activation(t_e[:], cell_lin[:], AF.Exp,
                                     bias=cn2mr[:], scale=c2r[:])
            else:
                nc.vector.tensor_scalar(new_c[:], cell_lin[:],
                                        mrbc[:, 0:1], mrbc[:, 1:2],
                                        op0=ALU.subtract, op1=ALU.mult)
                nc.vector.scalar_tensor_tensor(new_c[:], new_c[:], 1.0,
                                               gc_sb[:], op0=ALU.mult,
                                               op1=ALU.mult)
                nc.vector.scalar_tensor_tensor(new_c[:], new_c[:], 1.0,
                                               bc_sb[:], op0=ALU.mult,
                                               op1=ALU.add)
                nc.scalar.activation(t_e[:], new_c[:], AF.Exp, scale=2.0)

            # new_h = o * tanh(.) = o - 2*o*rt
            nc.vector.tensor_scalar_add(t_e[:], t_e[:], 1.0)
            t_r = sb.tile([8, S], F32, name="t_r")
            recipA(t_r[:], t_e[:])
            ort = sb.tile([8, S], F32, name="ort")
            nc.vector.scalar_tensor_tensor(ort[:], t_r[:], 1.0, o_gv,
                                           op0=ALU.mult, op1=ALU.mult)
            new_h = sb.tile([8, S], F32, name="new_h")
            nc.vector.scalar_tensor_tensor(new_h[:], ort[:], -2.0, o_gv,
                                           op0=ALU.mult, op1=ALU.add)

            nc.sync.dma_start(out_c[:], new_c[:])
            nc.scalar.dma_start(out_h[:], new_h[:])

    nc.compile()
    return nc


def _tmaj(v):
    """[2048] vector -> [128,16] t-major sbuf image (sb[p,t] = v[t*128+p])."""
    return np.ascontiguousarray(v.reshape(T, 128).T)


def _ktiles(wT, cols):
    """wT: [K_in, out_cols] -> [128, (K_in/128)*cols] partition-major pack."""
    k_in = wT.shape[0]
    return np.ascontiguousarray(
        wT.reshape(k_in // 128, 128, cols).transpose(1, 0, 2).reshape(
            128, (k_in // 128) * cols))


def kernel(input, hiddens, cells, external,
           W_ai, W_attn, W_merge, W_iou, W_fi, W_fh,
           g_merge, b_merge, g_f, b_f, g_i, b_i, g_o, b_o, g_u, b_u,
           g_c, b_c):
    f32 = np.float32
    gs = [np.asarray(g, f32) for g in
          (g_merge, g_f, g_i, g_o, g_u, g_c)]
    bs = [np.asarray(b, f32) for b in
          (b_merge, b_f, b_i, b_o, b_u, b_c)]
    trivial = (all(np.all(g == 1.0) for g in gs)
               and all(np.all(b == 0.0) for b in bs))
    key = ("nc", trivial)
    if key not in _CACHE:
        _CACHE[key] = _build(trivial)
    nc = _CACHE[key]

    input = np.asarray(input, f32)
    hiddens = np.asarray(hiddens, f32)
    cells = np.asarray(cells, f32)
    external = np.asarray(external, f32)

    hTt = _ktiles(np.ascontiguousarray(hiddens.T), N).astype(NPBF)
    xT32 = _ktiles(np.tile(input[:, None], (1, N)), N).astype(NPBF)
    eT32 = _ktiles(np.tile(external[:, None], (1, N)), N).astype(NPBF)
    x1 = _tmaj(input).astype(NPBF)

    com = {
        "hT": hTt, "xT32": xT32, "eT32": eT32, "x1": x1,
        "gm": _tmaj(gs[0]), "bm": _tmaj(bs[0]),
        "gi8": gs[2].reshape(8, S), "bi8": bs[2].reshape(8, S),
        "go8": gs[3].reshape(8, S), "bo8": bs[3].reshape(8, S),
        "gu8": gs[4].reshape(8, S), "bu8": bs[4].reshape(8, S),
        "gc8": gs[5].reshape(8, S), "bc8": bs[5].reshape(8, S),
        "ones8": np.ones((8, 1), f32), "ones32": np.ones((N, 1), f32),
        "ones128": np.ones((128, 1), f32),
        "onesr": np.ones((1, 128), f32),
    }

    Wf_cat = np.concatenate([W_fh, W_fi], axis=1)              # [H, 4096]
    in_maps = []
    for c in range(NC):
        r = slice(c * S, (c + 1) * S)
        iou_rows = np.concatenate(
            [W_iou[g * H + c * S:g * H + (c + 1) * S, :] for g in range(3)],
            axis=0)                                            # [768, 4096]
        m = dict(com)
        m.update({
            "hTc": np.ascontiguousarray(
                hiddens.T[c * S:(c + 1) * S].reshape(2, 128, N)
                .transpose(1, 0, 2).reshape(128, 2 * N)).astype(NPBF),
            "cells_chunk": np.ascontiguousarray(cells[:, r]),
            "gf_rep": np.tile(gs[1][r], (N, 1)),
            "bf_rep": np.tile(bs[1][r], (N, 1)),
            "wattn_rep": np.tile(np.asarray(W_attn, f32)[0, r], (N, 1)),
            "watsum": np.full((N, 1), np.asarray(W_attn, f32)[0, r].sum(),
                              f32),
            "wf": _ktiles(np.ascontiguousarray(Wf_cat[r].T), S).astype(NPBF),
            "wai": _ktiles(np.ascontiguousarray(W_ai[r].T), S).astype(NPBF),
            "wmg": _ktiles(np.ascontiguousarray(W_merge[:, r].T),
                           H).astype(NPBF),
            "wiou": _ktiles(np.ascontiguousarray(iou_rows.T),
                            3 * S).astype(NPBF),
        })
        in_maps.append({k: (np.ascontiguousarray(v) if v.dtype == NPBF
                            else np.ascontiguousarray(v, f32))
                        for k, v in m.items()})

    res = run_bass_kernel_spmd(nc, in_maps, core_ids=list(range(NC)))
    _CACHE["last_results"] = res
    r0 = res.results[0]
    new_h = r0["out_h"].reshape(H).astype(f32)
    new_c = r0["out_c"].reshape(H).astype(f32)
    return new_h, new_c


# revision 16
# speedup vs baseline: 1.2015x; 1.0349x over previous
"""AttentiveChildSumTreeLSTMCell on 8 Trainium2 NeuronCores.

Tensor-parallel: column-parallel f/attention/iou linears (hidden dim sharded
8 ways), row-parallel merge linear.  Collectives: two zero-dependency warmup
AllGathers (absorb communicator init + first-collective software setup),
AllGather of partial attention logits, AllReduce of merge-linear partials +
f LayerNorm stats, AllGather of iou/forget*cell chunks.  Matmul operands
are bf16; accumulation and all norm/gate math stays fp32.

All activations use a single ACT table set (ln+exp, loaded once): sigmoid
and tanh are computed via exp + DVE fast-reciprocal, LayerNorm rstd via
exp(-0.5*ln(var+eps)).  When all LayerNorm gains are 1 and biases 0 (the
common case, verified at runtime), the (x-mean)*rstd normalization is
folded into the exp activation's per-partition scale/bias operands.  The
gpsimd queue carries only collective triggers; the final gate math runs in
a [8, 256] chunk layout read straight from the AllGather result.
"""

import sys

for _p in ("/opt/trn_rl_repo",):
    if _p not in sys.path:
        sys.path.insert(0, _p)

import ml_dtypes
import numpy as np

import concourse.bacc as bacc
import concourse.mybir as mybir
import concourse.tile as tile
from concourse.bass_utils import run_bass_kernel_spmd
from concourse.tile_rust import add_dep_helper

F32 = mybir.dt.float32
BF16 = mybir.dt.bfloat16
AF = mybir.ActivationFunctionType
ALU = mybir.AluOpType
NPBF = ml_dtypes.bfloat16

H = 2048
N = 32
NC = 8
S = H // NC           # 256: per-core chunk of every sharded dim
T = H // 128          # 16 tiles of 128 along a 2048 dim
KT = 32               # K-tiles along the 4096 contraction dims
EPS = 1e-5
INV_H = 1.0 / H

# index of the ln+exp activation-function set in act_info.json
LN_EXP_SET = 6

_CACHE = {}


def _build(trivial_ln):
    nc = bacc.Bacc(None, target_bir_lowering=False, debug=False, num_devices=NC)

    def din(name, shape, dt=F32):
        return nc.dram_tensor(name, list(shape), dt, kind="ExternalInput")

    # ---- per-core DRAM inputs (SPMD: same shapes on every core) ----
    hT = din("hT", (128, T * N), BF16)
    xT32 = din("xT32", (128, T * N), BF16)
    eT32 = din("eT32", (128, T * N), BF16)
    x1 = din("x1", (128, T), BF16)
    hTc = din("hTc", (128, 2 * N), BF16)
    cells_chunk = din("cells_chunk", (N, S))
    gf_rep = din("gf_rep", (N, S))
    bf_rep = din("bf_rep", (N, S))
    wattn_rep = din("wattn_rep", (N, S))
    watsum = din("watsum", (N, 1))
    gm = din("gm", (128, T))
    bm = din("bm", (128, T))
    gi8 = din("gi8", (8, S))
    bi8 = din("bi8", (8, S))
    go8 = din("go8", (8, S))
    bo8 = din("bo8", (8, S))
    gu8 = din("gu8", (8, S))
    bu8 = din("bu8", (8, S))
    gc8 = din("gc8", (8, S))
    bc8 = din("bc8", (8, S))
    ones8 = din("ones8", (8, 1))
    ones32 = din("ones32", (N, 1))
    ones128 = din("ones128", (128, 1))
    onesr = din("onesr", (1, 128))
    wf = din("wf", (128, KT * S), BF16)        # [W_fh | W_fi]^T chunk
    wai = din("wai", (128, KT * S), BF16)      # W_ai^T chunk
    wmg = din("wmg", (128, 2 * H), BF16)       # W_merge^T in-chunk
    wiou = din("wiou", (128, KT * 3 * S), BF16)  # W_iou^T chunk (i|o|u cols)

    out_h = nc.dram_tensor("out_h", [8, S], F32, kind="ExternalOutput")
    out_c = nc.dram_tensor("out_c", [8, S], F32, kind="ExternalOutput")

    with tile.TileContext(nc) as tc:
        with (
            tc.tile_pool(name="sb", bufs=1) as sb,
            tc.tile_pool(name="ps", bufs=1, space="PSUM") as ps,
            tc.tile_pool(name="dram", bufs=1, space="DRAM") as dram,
        ):
            # ---- warmup collectives: zero deps, trigger at t~0 -----------
            # Contents are irrelevant; they pull communicator init and
            # per-kind first-collective software setup off the critical path.
            warm_in = dram.tile([1, 16], F32, name="warm_in")
            warm_out = dram.tile([8, 16], F32, name="warm_out")
            with tc.high_priority():
                nc.gpsimd.collective_compute(
                    "AllGather", ALU.bypass,
                    replica_groups=[list(range(NC))],
                    ins=[warm_in.opt()], outs=[warm_out.opt()])
                # single ACT table load for the whole kernel (ln+exp set)
                nc.scalar.add_instruction(mybir.InstLoadActFuncSet(
                    name=f"I-{nc.next_id()}", ins=[], outs=[],
                    act_func_set_id=LN_EXP_SET))

            # ------- small resident loads (scalar queue; SP is weights) ---
            def load(t_dram, shape, dt=F32):
                t_sb = sb.tile(shape, dt, name=t_dram.name + "_sb")
                nc.scalar.dma_start(t_sb[:], t_dram[:])
                return t_sb

            hT_sb = load(hT, [128, T, N], BF16)
            xT32_sb = load(xT32, [128, T, N], BF16)
            eT32_sb = load(eT32, [128, T, N], BF16)
            x1_sb = load(x1, [128, T], BF16)
            hTc_sb = load(hTc, [128, 2, N], BF16)
            cells_sb = load(cells_chunk, [N, S])
            wat_sb = load(wattn_rep, [N, S])
            wsum_sb = load(watsum, [N, 1])
            ones8_sb = load(ones8, [8, 1])
            ones32_sb = load(ones32, [N, 1])
            ones128_sb = load(ones128, [128, 1])
            onesr_sb = load(onesr, [1, 128])
            if not trivial_ln:
                gf_sb = load(gf_rep, [N, S])
                bf_sb = load(bf_rep, [N, S])
                gm_sb = load(gm, [128, T])
                bm_sb = load(bm, [128, T])
                gi_sb = load(gi8, [8, S])
                bi_sb = load(bi8, [8, S])
                go_sb = load(go8, [8, S])
                bo_sb = load(bo8, [8, S])
                gu_sb = load(gu8, [8, S])
                bu_sb = load(bu8, [8, S])
                gc_sb = load(gc8, [8, S])
                bc_sb = load(bc8, [8, S])

            eps1 = sb.tile([1, 1], F32, name="eps1")
            nc.vector.memset(eps1[:], EPS)
            epsN = sb.tile([N, 1], F32, name="epsN")
            nc.vector.memset(epsN[:], EPS)

            def recipA(out, in_):
                nc.vector.reciprocal_approx_fast(out=out, in_=in_)

            # ---------------- weight streaming DMAs (SP, ordered) ---------
            wai_sb = sb.tile([128, KT * S], BF16, name="wai_sb")
            wf_sb = sb.tile([128, KT * S], BF16, name="wf_sb")
            wiou_sb = sb.tile([128, KT * 3 * S], BF16, name="wiou_sb")
            wmg_sb = sb.tile([128, 2 * H], BF16, name="wmg_sb")

            wdmas = []
            for k in range(2):  # wai: 2 x 1MB
                wdmas.append(nc.sync.dma_start(
                    wai_sb[:, k * 4096:(k + 1) * 4096],
                    wai[:, k * 4096:(k + 1) * 4096]))
            for k in range(2):  # wf: 2 x 1MB
                wdmas.append(nc.sync.dma_start(
                    wf_sb[:, k * 4096:(k + 1) * 4096],
                    wf[:, k * 4096:(k + 1) * 4096]))
            for k in range(3):  # wiou x-half: 3 x 1MB
                wdmas.append(nc.sync.dma_start(
                    wiou_sb[:, k * 4096:(k + 1) * 4096],
                    wiou[:, k * 4096:(k + 1) * 4096]))
            wdmas.append(nc.sync.dma_start(wmg_sb[:], wmg[:]))  # 1MB
            for k in range(3, 6):  # wiou mh-half: 3 x 1MB
                wdmas.append(nc.sync.dma_start(
                    wiou_sb[:, k * 4096:(k + 1) * 4096],
                    wiou[:, k * 4096:(k + 1) * 4096]))
            # chain three-deep: keeps arrival order without strangling BW
            for i in range(3, len(wdmas)):
                add_dep_helper(wdmas[i].ins, wdmas[i - 3].ins, sync=True,
                               reason="weight DMA arrival order")

            # ---------------- attention: ai -> partial logits -------------
            ps_ai = ps.tile([N, S], F32, name="ps_ai", tag="pA")
            for kt in range(KT):
                act = hT_sb if kt < T else eT32_sb
                nc.tensor.matmul(ps_ai[:], act[:, kt % T, :],
                                 wai_sb[:, kt * S:(kt + 1) * S],
                                 start=(kt == 0), stop=(kt == KT - 1))
            # logits = sum_j wat * tanh(ai); tanh(w) = 2/(1+exp(-2w)) - 1
            # lg = 2 * sum(wat * r) - sum(wat),  r = 1/(1+exp(-2w))
            ai_e = sb.tile([N, S], F32, name="ai_e")
            nc.scalar.activation(ai_e[:], ps_ai[:], AF.Exp, scale=-2.0)
            ai_d = sb.tile([N, S], F32, name="ai_d")
            nc.vector.tensor_scalar_add(ai_d[:], ai_e[:], 1.0)
            ai_r = sb.tile([N, S], F32, name="ai_r")
            recipA(ai_r[:], ai_d[:])
            aw = sb.tile([N, S], F32, name="aw")
            asum = sb.tile([N, 1], F32, name="asum")
            nc.vector.scalar_tensor_tensor(aw[:], ai_r[:], 1.0, wat_sb[:],
                                           op0=ALU.mult, op1=ALU.mult,
                                           accum_out=asum[:])
            lg_sb = sb.tile([N, 1], F32, name="lg_sb")
            nc.vector.tensor_scalar(lg_sb[:], asum[:], 2.0, wsum_sb[:],
                                    op0=ALU.mult, op1=ALU.subtract)

            # ---------------- AG1: partial logits -------------------------
            ag1_in = dram.tile([1, N], F32, name="ag1_in")
            ag1_out_t = nc.dram_tensor("ag1_out", [8, N], F32,
                                       kind="Internal", addr_space="Shared")
            nc.scalar.dma_start(
                ag1_in[0, :].rearrange("(p one) -> p one", one=1), lg_sb[:])
            nc.gpsimd.collective_compute(
                "AllGather", ALU.bypass,
                replica_groups=[list(range(NC))],
                ins=[ag1_in.opt()], outs=[ag1_out_t[:]])

            # ---------------- f_lin + per-child stats (speculative) -------
            ps_f = ps.tile([N, S], F32, name="ps_f", tag="pG")
            for kt in range(KT):
                act = hT_sb if kt < T else xT32_sb
                nc.tensor.matmul(ps_f[:], act[:, kt % T, :],
                                 wf_sb[:, kt * S:(kt + 1) * S],
                                 start=(kt == 0), stop=(kt == KT - 1))
            f_lin_sb = sb.tile([N, S], F32, name="f_lin_sb")
            fst2 = sb.tile([N, 2], F32, name="fst2")
            fsq_scr = sb.tile([N, S], F32, name="fsq_scr")
            nc.vector.tensor_copy(f_lin_sb[:], ps_f[:])
            nc.vector.tensor_reduce(fst2[:, 0:1], f_lin_sb[:],
                                    mybir.AxisListType.X, ALU.add)
            nc.vector.scalar_tensor_tensor(fsq_scr[:], f_lin_sb[:], 1.0,
                                           f_lin_sb[:], op0=ALU.mult,
                                           op1=ALU.mult,
                                           accum_out=fst2[:, 1:2])

            # speculative per-child merge projections (pre-attention):
            # M[p, t, n] = sum_in W_merge[t*128+p, in] * h[n, in], in-chunk
            ps_M = ps.tile([128, T, N], F32, name="ps_M", tag="pD")
            for t in range(T):
                for s in range(2):
                    nc.tensor.matmul(
                        ps_M[:, t, :],
                        wmg_sb[:, s * H + t * 128: s * H + (t + 1) * 128],
                        hTc_sb[:, s, :],
                        start=(s == 0), stop=(s == 1))

            ps_iou = ps.tile([1, 3 * S], F32, name="ps_iou", tag="pIOU")
            nslices = ((0, 512), (512, 768))

            def iou_mm(kt, start, stop):
                lhs = (x1_sb[:, kt:kt + 1] if kt < T
                       else mh_bf[:, kt - T:kt - T + 1])
                for c0, c1 in nslices:
                    nc.tensor.matmul(ps_iou[:, c0:c1],
                                     lhs, wiou_sb[:, kt * 768 + c0:
                                                  kt * 768 + c1],
                                     start=start, stop=stop)

            # ---------------- post-AG1: softmax + merge partials ----------
            ag1_sb = sb.tile([8, N], F32, name="ag1_sb")
            nc.sync.dma_start(ag1_sb[:], ag1_out_t[:])
            ps_l2r = ps.tile([1, N], F32, name="ps_l2r", tag="pB")
            nc.tensor.matmul(ps_l2r[:], ones8_sb[:], ag1_sb[:],
                             start=True, stop=True)
            exps_row = sb.tile([1, N], F32, name="exps_row")
            # softmax without max-subtraction or normalization: the scale
            # cancels inside the merge LayerNorm
            nc.scalar.activation(exps_row[:], ps_l2r[:], AF.Exp)
            ps_eb = ps.tile([128, N], F32, name="ps_eb", tag="pH")
            nc.tensor.matmul(ps_eb[:], onesr_sb[:], exps_row[:],
                             start=True, stop=True)

            # merge-linear partials: one multiply + one reduce
            exps_b = sb.tile([128, N], F32, name="exps_b")
            nc.vector.tensor_copy(exps_b[:], ps_eb[:])
            mp_sb = sb.tile([128, T], F32, name="mp_sb")
            mp_scr3 = sb.tile([128, T, N], F32, name="mp_scr3")
            eb3 = exps_b[:].rearrange("p (one n) -> p one n",
                                      one=1).to_broadcast((128, T, N))
            nc.vector.scalar_tensor_tensor(mp_scr3[:], ps_M[:], 1.0, eb3,
                                           op0=ALU.mult, op1=ALU.mult)
            nc.vector.tensor_reduce(mp_sb[:], mp_scr3[:],
                                    mybir.AxisListType.X, ALU.add)

            # iou x-half scheduled in the AR2 wait window: a junk pass
            # first (keeps the tensor-engine clock high through the wait),
            # then the real accumulation, so the AR2-gated mh-half block
            # starts on a hot PE.
            ps_junk = ps.tile([1, 512], F32, name="ps_junk", tag="pG")
            for kt in range(T):
                for rep in range(2):
                    nc.tensor.matmul(ps_junk[:],
                                     x1_sb[:, kt:kt + 1],
                                     wiou_sb[:, kt * 768:kt * 768 + 512],
                                     start=(kt == 0 and rep == 0),
                                     stop=(kt == T - 1 and rep == 1))
            for kt in range(T):          # real x half
                iou_mm(kt, kt == 0, False)

            # ---------------- AR2: merge partials + f stats ---------------
            ar2_in = dram.tile([1, H + 2 * N], BF16, name="ar2_in")
            ar2_out = nc.dram_tensor("ar2_out", [1, H + 2 * N], BF16,
                                     kind="Internal", addr_space="Shared")
            mp_bf = sb.tile([128, T], BF16, name="mp_bf")
            nc.vector.tensor_copy(mp_bf[:], mp_sb[:])
            nc.sync.dma_start(
                ar2_in[0, 0:H].rearrange("(p t) -> p t", p=128), mp_bf[:])
            fst_bf = sb.tile([N, 2], BF16, name="fst_bf")
            nc.vector.tensor_copy(fst_bf[:], fst2[:])
            nc.scalar.dma_start(
                ar2_in[0, H:H + 2 * N].rearrange("(p s) -> p s", s=2),
                fst_bf[:])
            nc.gpsimd.collective_compute(
                "AllReduce", ALU.add,
                replica_groups=[list(range(NC))],
                ins=[ar2_in.opt()], outs=[ar2_out[:]])

            # parallel readbacks on separate queues
            ml_sb = sb.tile([128, T], BF16, name="ml_sb")
            nc.sync.dma_start(
                ml_sb[:], ar2_out[0, 0:H].rearrange("(p t) -> p t", p=128))
            fst_t = sb.tile([N, 2], BF16, name="fst_t")
            nc.scalar.dma_start(
                fst_t[:], ar2_out[0, H:H + 2 * N].rearrange("(p s) -> p s",
                                                            s=2))

            # ---------------- merge-hidden LayerNorm + tanh ---------------
            st2 = sb.tile([128, 2], F32, name="st2")
            sq_scr = sb.tile([128, T], F32, name="sq_scr")
            nc.vector.tensor_reduce(st2[:, 0:1], ml_sb[:],
                                    mybir.AxisListType.X, ALU.add)
            nc.vector.scalar_tensor_tensor(sq_scr[:], ml_sb[:], 1.0,
                                           ml_sb[:], op0=ALU.mult,
                                           op1=ALU.mult,
                                           accum_out=st2[:, 1:2])
            ps_st = ps.tile([1, 2], F32, name="ps_st", tag="pB")
            nc.tensor.matmul(ps_st[:], ones128_sb[:], st2[:],
                             start=True, stop=True)
            mr = sb.tile([1, 2], F32, name="mr")
            var = sb.tile([1, 1], F32, name="mvar")
            lnv = sb.tile([1, 1], F32, name="mlnv")
            nc.vector.tensor_scalar_mul(mr[:], ps_st[:], INV_H)
            nc.vector.scalar_tensor_tensor(var[:], mr[:, 0:1], 1.0,
                                           mr[:, 0:1], op0=ALU.mult,
                                           op1=ALU.mult)
            nc.vector.tensor_sub(var[:], mr[:, 1:2], var[:])
            nc.scalar.activation(lnv[:], var[:], AF.Ln, bias=eps1[:])
            nc.scalar.activation(mr[:, 1:2], lnv[:], AF.Exp, scale=-0.5)
            ps_bc = ps.tile([128, 2], F32, name="ps_bc", tag="pH")
            nc.tensor.matmul(ps_bc[:], onesr_sb[:], mr[:],
                             start=True, stop=True)
            mrbm = sb.tile([128, 2], F32, name="mrbm")
            nc.vector.tensor_copy(mrbm[:], ps_bc[:])
            mh_e = sb.tile([128, T], F32, name="mh_e")
            if trivial_ln:
                # exp(2*(ml - m)*r) folded into ACT scale/bias:
                # scale = 2r, bias = -2*m*r  (per-partition broadcasts)
                s2r = sb.tile([128, 1], F32, name="s2r")
                nc.vector.tensor_scalar_mul(s2r[:], mrbm[:, 1:2], 2.0)
                b2mr = sb.tile([128, 1], F32, name="b2mr")
                nc.vector.scalar_tensor_tensor(b2mr[:], s2r[:], -1.0,
                                               mrbm[:, 0:1], op0=ALU.mult,
                                               op1=ALU.mult)
                nc.scalar.activation(mh_e[:], ml_sb[:], AF.Exp,
                                     bias=b2mr[:], scale=s2r[:])
            else:
                mh_t1 = sb.tile([128, T], F32, name="mh_t1")
                nc.vector.tensor_scalar(mh_t1[:], ml_sb[:], mrbm[:, 0:1],
                                        mrbm[:, 1:2], op0=ALU.subtract,
                                        op1=ALU.mult)
                nc.vector.scalar_tensor_tensor(mh_t1[:], mh_t1[:], 1.0,
                                               gm_sb[:], op0=ALU.mult,
                                               op1=ALU.mult)
                nc.vector.scalar_tensor_tensor(mh_t1[:], mh_t1[:], 1.0,
                                               bm_sb[:], op0=ALU.mult,
                                               op1=ALU.add)
                nc.scalar.activation(mh_e[:], mh_t1[:], AF.Exp, scale=2.0)
            nc.vector.tensor_scalar_add(mh_e[:], mh_e[:], 1.0)
            mh_r = sb.tile([128, T], F32, name="mh_r")
            recipA(mh_r[:], mh_e[:])
            mh_bf = sb.tile([128, T], BF16, name="mh_bf")
            nc.vector.tensor_scalar(mh_bf[:], mh_r[:], -2.0, 1.0,
                                    op0=ALU.mult, op1=ALU.add)

            # ---------------- iou mh-half (post-AR2 PE work) --------------
            for kt in range(T, KT):
                iou_mm(kt, False, kt == KT - 1)

            # ---------------- f gate + fc = sum_n f*cells -----------------
            fmr = sb.tile([N, 2], F32, name="fmr")
            flnv = sb.tile([N, 1], F32, name="flnv")
            fvar = sb.tile([N, 1], F32, name="fvar")
            nc.vector.tensor_scalar_mul(fmr[:], fst_t[:], INV_H)
            nc.vector.scalar_tensor_tensor(fvar[:], fmr[:, 0:1], 1.0,
                                           fmr[:, 0:1], op0=ALU.mult,
                                           op1=ALU.mult)
            nc.vector.tensor_sub(fvar[:], fmr[:, 1:2], fvar[:])
            nc.scalar.activation(flnv[:], fvar[:], AF.Ln, bias=epsN[:])
            nc.scalar.activation(fmr[:, 1:2], flnv[:], AF.Exp, scale=-0.5)
            f_e = sb.tile([N, S], F32, name="f_e")
            if trivial_ln:
                # sig((x-m)*r): exp(-(x-m)*r) via scale=-r, bias=m*r
                fnr = sb.tile([N, 1], F32, name="fnr")
                nc.vector.tensor_scalar_mul(fnr[:], fmr[:, 1:2], -1.0)
                fmrb = sb.tile([N, 1], F32, name="fmrb")
                nc.vector.scalar_tensor_tensor(fmrb[:], fnr[:], -1.0,
                                               fmr[:, 0:1], op0=ALU.mult,
                                               op1=ALU.mult)
                nc.scalar.activation(f_e[:], f_lin_sb[:], AF.Exp,
                                     bias=fmrb[:], scale=fnr[:])
            else:
                ft = sb.tile([N, S], F32, name="ft")
                nc.vector.tensor_scalar(ft[:], f_lin_sb[:], fmr[:, 0:1],
                                        fmr[:, 1:2], op0=ALU.subtract,
                                        op1=ALU.mult)
                nc.vector.scalar_tensor_tensor(ft[:], ft[:], 1.0, gf_sb[:],
                                               op0=ALU.mult, op1=ALU.mult)
                nc.vector.scalar_tensor_tensor(ft[:], ft[:], 1.0, bf_sb[:],
                                               op0=ALU.mult, op1=ALU.add)
                nc.scalar.activation(f_e[:], ft[:], AF.Exp, scale=-1.0)
            nc.vector.tensor_scalar_add(f_e[:], f_e[:], 1.0)
            f_r = sb.tile([N, S], F32, name="f_r")
            recipA(f_r[:], f_e[:])
            fprod = sb.tile([N, S], F32, name="fprod")
            nc.vector.scalar_tensor_tensor(fprod[:], f_r[:], 1.0,
                                           cells_sb[:], op0=ALU.mult,
                                           op1=ALU.mult)
            ps_fc = ps.tile([1, S], F32, name="ps_fc", tag="pC")
            nc.tensor.matmul(ps_fc[:], ones32_sb[:], fprod[:],
                             start=True, stop=True)

            # ---------------- AG3: iou chunk + fc chunk -------------------
            ag3_in = dram.tile([1, 4 * S], BF16, name="ag3_in")
            ag3_out = nc.dram_tensor("ag3_out", [8, 4 * S], BF16,
                                     kind="Internal", addr_space="Shared")
            fc_bf = sb.tile([1, S], BF16, name="fc_bf")
            nc.scalar.activation(fc_bf[:], ps_fc[:], AF.Copy)
            nc.scalar.dma_start(ag3_in[0, 3 * S:4 * S].rearrange(
                "(a f) -> a f", a=1), fc_bf[:])
            ag3_stage = sb.tile([1, 3 * S], BF16, name="ag3_stage")
            nc.vector.tensor_copy(ag3_stage[:, 0:512], ps_iou[:, 0:512])
            nc.scalar.activation(ag3_stage[:, 512:3 * S],
                                 ps_iou[:, 512:3 * S], AF.Copy)
            nc.sync.dma_start(ag3_in[0, 0:3 * S].rearrange(
                "(a f) -> a f", a=1), ag3_stage[:])
            nc.gpsimd.collective_compute(
                "AllGather", ALU.bypass,
                replica_groups=[list(range(NC))],
                ins=[ag3_in.opt()], outs=[ag3_out[:]])

            # single contiguous readback: [8, 1024] chunk-major
            ag3_sb = sb.tile([8, 4 * S], BF16, name="ag3_sb")
            nc.sync.dma_start(ag3_sb[:], ag3_out[:])
            i_l = ag3_sb[:, 0 * S:1 * S]
            o_l = ag3_sb[:, 1 * S:2 * S]
            u_l = ag3_sb[:, 2 * S:3 * S]
            fc_v = ag3_sb[:, 3 * S:4 * S]

            # LayerNorm stats for i/o/u: sums in one DVE reduce, squared
            # sums on the ACT engine (Square + accumulate), in parallel
            st6 = sb.tile([8, 6], F32, name="st6")
            iou3 = ag3_sb[:, 0:3 * S].rearrange("p (g f) -> p g f", g=3)
            nc.vector.tensor_reduce(st6[:, 0:3], iou3,
                                    mybir.AxisListType.X, ALU.add)
            sqa_scr = sb.tile([8, S], F32, name="sqa_scr")
            for v, vec in enumerate((i_l, o_l, u_l)):
                nc.scalar.activation(sqa_scr[:], vec, AF.Square,
                                     accum_out=st6[:, 3 + v:4 + v])
            ps_st6 = ps.tile([1, 6], F32, name="ps_st6", tag="pA")
            nc.tensor.matmul(ps_st6[:], ones8_sb[:], st6[:],
                             start=True, stop=True)
            mr6 = sb.tile([1, 6], F32, name="mr6")
            var3 = sb.tile([1, 3], F32, name="var3")
            lnv3 = sb.tile([1, 3], F32, name="lnv3")
            nc.vector.tensor_scalar_mul(mr6[:], ps_st6[:], INV_H)
            nc.vector.scalar_tensor_tensor(var3[:], mr6[:, 0:3], 1.0,
                                           mr6[:, 0:3], op0=ALU.mult,
                                           op1=ALU.mult)
            nc.vector.tensor_sub(var3[:], mr6[:, 3:6], var3[:])
            nc.scalar.activation(lnv3[:], var3[:], AF.Ln, bias=eps1[:])
            nc.scalar.activation(mr6[:, 3:6], lnv3[:], AF.Exp, scale=-0.5)
            ps_b6 = ps.tile([8, 6], F32, name="ps_b6", tag="pB")
            nc.tensor.matmul(ps_b6[:], onesr_sb[:, 0:8], mr6[:],
                             start=True, stop=True)
            mrb6 = sb.tile([8, 6], F32, name="mrb6")
            nc.vector.tensor_copy(mrb6[:], ps_b6[:])

            if trivial_ln:
                # negated / doubled per-partition scale-bias precomputes
                nr3 = sb.tile([8, 3], F32, name="nr3")
                nc.vector.tensor_scalar_mul(nr3[:], mrb6[:, 3:6], -1.0)
                mr3 = sb.tile([8, 3], F32, name="mr3")
                nc.vector.scalar_tensor_tensor(mr3[:], mrb6[:, 0:3], 1.0,
                                               mrb6[:, 3:6], op0=ALU.mult,
                                               op1=ALU.mult)
                p2ru = sb.tile([8, 1], F32, name="p2ru")
                nc.vector.tensor_scalar_mul(p2ru[:], mrb6[:, 5:6], 2.0)
                n2mru = sb.tile([8, 1], F32, name="n2mru")
                nc.vector.tensor_scalar_mul(n2mru[:], mr3[:, 2:3], -2.0)

                e3 = sb.tile([8, 3 * S], F32, name="e3")
                nc.scalar.activation(e3[:, 0:S], i_l, AF.Exp,
                                     bias=mr3[:, 0:1], scale=nr3[:, 0:1])
                nc.scalar.activation(e3[:, S:2 * S], o_l, AF.Exp,
                                     bias=mr3[:, 1:2], scale=nr3[:, 1:2])
                nc.scalar.activation(e3[:, 2 * S:3 * S], u_l, AF.Exp,
                                     bias=n2mru[:], scale=p2ru[:])
                nc.vector.tensor_scalar_add(e3[:], e3[:], 1.0)
                r3g = sb.tile([8, 3 * S], F32, name="r3g")
                recipA(r3g[:], e3[:])
                i_g = r3g[:, 0:S]
                o_g = r3g[:, S:2 * S]
                u_r3 = r3g[:, 2 * S:3 * S]
            else:
                def gate_ln(vec, v, g_t, b_t, nm):
                    t1 = sb.tile([8, S], F32, name=nm + "_t1")
                    nc.vector.tensor_scalar(t1[:], vec,
                                            mrb6[:, v:v + 1],
                                            mrb6[:, 3 + v:4 + v],
                                            op0=ALU.subtract, op1=ALU.mult)
                    nc.vector.scalar_tensor_tensor(t1[:], t1[:], 1.0,
                                                   g_t[:], op0=ALU.mult,
                                                   op1=ALU.mult)
                    nc.vector.scalar_tensor_tensor(t1[:], t1[:], 1.0,
                                                   b_t[:], op0=ALU.mult,
                                                   op1=ALU.add)
                    return t1

                yi = gate_ln(i_l, 0, gi_sb, bi_sb, "yi")
                yo = gate_ln(o_l, 1, go_sb, bo_sb, "yo")
                yu = gate_ln(u_l, 2, gu_sb, bu_sb, "yu")

                def sig(y, nm):
                    e = sb.tile([8, S], F32, name=nm + "_e")
                    nc.scalar.activation(e[:], y[:], AF.Exp, scale=-1.0)
                    nc.vector.tensor_scalar_add(e[:], e[:], 1.0)
                    r = sb.tile([8, S], F32, name=nm + "_r")
                    recipA(r[:], e[:])
                    return r

                i_g = sig(yi, "ig")
                o_g = sig(yo, "og")
                u_e = sb.tile([8, S], F32, name="u_e")
                nc.scalar.activation(u_e[:], yu[:], AF.Exp, scale=2.0)

            if trivial_ln:
                u_rv = u_r3
                i_gv, o_gv = i_g, o_g
            else:
                nc.vector.tensor_scalar_add(u_e[:], u_e[:], 1.0)
                u_r = sb.tile([8, S], F32, name="u_r")
                recipA(u_r[:], u_e[:])
                u_rv = u_r[:]
                i_gv, o_gv = i_g[:], o_g[:]
            # cell_lin = i*u + fc = i - 2*i*ru + fc
            iru = sb.tile([8, S], F32, name="iru")
            nc.vector.scalar_tensor_tensor(iru[:], u_rv, 1.0, i_gv,
                                           op0=ALU.mult, op1=ALU.mult)
            cell_lin = sb.tile([8, S], F32, name="cell_lin")
            nc.vector.scalar_tensor_tensor(cell_lin[:], iru[:], -2.0,
                                           i_gv, op0=ALU.mult,
                                           op1=ALU.add)
            nc.vector.scalar_tensor_tensor(cell_lin[:], cell_lin[:], 1.0,
                                           fc_v, op0=ALU.mult, op1=ALU.add)

            # cell LayerNorm
            cst = sb.tile([8, 2], F32, name="cst")
            csq = sb.tile([8, S], F32, name="csq")
            nc.vector.tensor_reduce(cst[:, 0:1], cell_lin[:],
                                    mybir.AxisListType.X, ALU.add)
            nc.scalar.activation(csq[:], cell_lin[:], AF.Square,
                                 accum_out=cst[:, 1:2])
            ps_cst = ps.tile([1, 2], F32, name="ps_cst", tag="pC")
            nc.tensor.matmul(ps_cst[:], ones8_sb[:], cst[:],
                             start=True, stop=True)
            cmr = sb.tile([1, 2], F32, name="cmr")
            cvar = sb.tile([1, 1], F32, name="cvar")
            clnv = sb.tile([1, 1], F32, name="clnv")
            nc.vector.tensor_scalar_mul(cmr[:], ps_cst[:], INV_H)
            nc.vector.scalar_tensor_tensor(cvar[:], cmr[:, 0:1], 1.0,
                                           cmr[:, 0:1], op0=ALU.mult,
                                           op1=ALU.mult)
            nc.vector.tensor_sub(cvar[:], cmr[:, 1:2], cvar[:])
            nc.scalar.activation(clnv[:], cvar[:], AF.Ln, bias=eps1[:])
            nc.scalar.activation(cmr[:, 1:2], clnv[:], AF.Exp, scale=-0.5)
            ps_cb = ps.tile([8, 2], F32, name="ps_cb", tag="pD")
            nc.tensor.matmul(ps_cb[:], onesr_sb[:, 0:8], cmr[:],
                             start=True, stop=True)
            mrbc = sb.tile([8, 2], F32, name="mrbc")
            nc.vector.tensor_copy(mrbc[:], ps_cb[:])
            new_c = sb.tile([8, S], F32, name="new_c")
            t_e = sb.tile([8, S], F32, name="t_e")
            if trivial_ln:
                nc.vector.tensor_scalar(new_c[:], cell_lin[:],
                                        mrbc[:, 0:1], mrbc[:, 1:2],
                                        op0=ALU.subtract, op1=ALU.mult)
                # tanh arg: exp(2*(cell-m)*r) via scale=2r, bias=-2mr
                c2r = sb.tile([8, 1], F32, name="c2r")
                nc.vector.tensor_scalar_mul(c2r[:], mrbc[:, 1:2], 2.0)
                cn2mr = sb.tile([8, 1], F32, name="cn2mr")
                nc.vector.scalar_tensor_tensor(cn2mr[:], c2r[:], -1.0,
                                               mrbc[:, 0:1], op0=ALU.mult,
                                               op1=ALU.mult)
                nc.scalar.activation(t_e[:], cell_lin[:], AF.Exp,
                                     bias=cn2mr[:], scale=c2r[:])
            else:
                nc.vector.tensor_scalar(new_c[:], cell_lin[:],
                                        mrbc[:, 0:1], mrbc[:, 1:2],
                                        op0=ALU.subtract, op1=ALU.mult)
                nc.vector.scalar_tensor_tensor(new_c[:], new_c[:], 1.0,
                                               gc_sb[:], op0=ALU.mult,
                                               op1=ALU.mult)
                nc.vector.scalar_tensor_tensor(new_c[:], new_c[:], 1.0,
                                               bc_sb[:], op0=ALU.mult,
                                               op1=ALU.add)
                nc.scalar.activation(t_e[:], new_c[:], AF.Exp, scale=2.0)

            # new_h = o * tanh(.) = o - 2*o*rt
            nc.vector.tensor_scalar_add(t_e[:], t_e[:], 1.0)
            t_r = sb.tile([8, S], F32, name="t_r")
            recipA(t_r[:], t_e[:])
            ort = sb.tile([8, S], F32, name="ort")
            nc.vector.scalar_tensor_tensor(ort[:], t_r[:], 1.0, o_gv,
                                           op0=ALU.mult, op1=ALU.mult)
            new_h = sb.tile([8, S], F32, name="new_h")
            nc.vector.scalar_tensor_tensor(new_h[:], ort[:], -2.0, o_gv,
                                           op0=ALU.mult, op1=ALU.add)

            nc.sync.dma_start(out_c[:], new_c[:])
            nc.scalar.dma_start(out_h[:], new_h[:])

    nc.compile()
    return nc


def _tmaj(v):
    """[2048] vector -> [128,16] t-major sbuf image (sb[p,t] = v[t*128+p])."""
    return np.ascontiguousarray(v.reshape(T, 128).T)


def _ktiles(wT, cols):
    """wT: [K_in, out_cols] -> [128, (K_in/128)*cols] partition-major pack."""
    k_in = wT.shape[0]
    return np.ascontiguousarray(
        wT.reshape(k_in // 128, 128, cols).transpose(1, 0, 2).reshape(
            128, (k_in // 128) * cols))


def kernel(input, hiddens, cells, external,
           W_ai, W_attn, W_merge, W_iou, W_fi, W_fh,
           g_merge, b_merge, g_f, b_f, g_i, b_i, g_o, b_o, g_u, b_u,
           g_c, b_c):
    f32 = np.float32
    gs = [np.asarray(g, f32) for g in
          (g_merge, g_f, g_i, g_o, g_u, g_c)]
    bs = [np.asarray(b, f32) for b in
          (b_merge, b_f, b_i, b_o, b_u, b_c)]
    trivial = (all(np.all(g == 1.0) for g in gs)
               and all(np.all(b == 0.0) for b in bs))
    key = ("nc", trivial)
    if key not in _CACHE:
        _CACHE[key] = _build(trivial)
    nc = _CACHE[key]

    input = np.asarray(input, f32)
    hiddens = np.asarray(hiddens, f32)
    cells = np.asarray(cells, f32)
    external = np.asarray(external, f32)

    hTt = _ktiles(np.ascontiguousarray(hiddens.T), N).astype(NPBF)
    xT32 = _ktiles(np.tile(input[:, None], (1, N)), N).astype(NPBF)
    eT32 = _ktiles(np.tile(external[:, None], (1, N)), N).astype(NPBF)
    x1 = _tmaj(input).astype(NPBF)

    com = {
        "hT": hTt, "xT32": xT32, "eT32": eT32, "x1": x1,
        "gm": _tmaj(gs[0]), "bm": _tmaj(bs[0]),
        "gi8": gs[2].reshape(8, S), "bi8": bs[2].reshape(8, S),
        "go8": gs[3].reshape(8, S), "bo8": bs[3].reshape(8, S),
        "gu8": gs[4].reshape(8, S), "bu8": bs[4].reshape(8, S),
        "gc8": gs[5].reshape(8, S), "bc8": bs[5].reshape(8, S),
        "ones8": np.ones((8, 1), f32), "ones32": np.ones((N, 1), f32),
        "ones128": np.ones((128, 1), f32),
        "onesr": np.ones((1, 128), f32),
    }

    Wf_cat = np.concatenate([W_fh, W_fi], axis=1)              # [H, 4096]
    in_maps = []
    for c in range(NC):
        r = slice(c * S, (c + 1) * S)
        iou_rows = np.concatenate(
            [W_iou[g * H + c * S:g * H + (c + 1) * S, :] for g in range(3)],
            axis=0)                                            # [768, 4096]
        m = dict(com)
        m.update({
            "hTc": np.ascontiguousarray(
                hiddens.T[c * S:(c + 1) * S].reshape(2, 128, N)
                .transpose(1, 0, 2).reshape(128, 2 * N)).astype(NPBF),
            "cells_chunk": np.ascontiguousarray(cells[:, r]),
            "gf_rep": np.tile(gs[1][r], (N, 1)),
            "bf_rep": np.tile(bs[1][r], (N, 1)),
            "wattn_rep": np.tile(np.asarray(W_attn, f32)[0, r], (N, 1)),
            "watsum": np.full((N, 1), np.asarray(W_attn, f32)[0, r].sum(),
                              f32),
            "wf": _ktiles(np.ascontiguousarray(Wf_cat[r].T), S).astype(NPBF),
            "wai": _ktiles(np.ascontiguousarray(W_ai[r].T), S).astype(NPBF),
            "wmg": _ktiles(np.ascontiguousarray(W_merge[:, r].T),
                           H).astype(NPBF),
            "wiou": _ktiles(np.ascontiguousarray(iou_rows.T),
                            3 * S).astype(NPBF),
        })
        in_maps.append({k: (np.ascontiguousarray(v) if v.dtype == NPBF
                            else np.ascontiguousarray(v, f32))
                        for k, v in m.items()})

    res = run_bass_kernel_spmd(nc, in_maps, core_ids=list(range(NC)))
    _CACHE["last_results"] = res
    r0 = res.results[0]
    new_h = r0["out_h"].reshape(H).astype(f32)
    new_c = r0["out_c"].reshape(H).astype(f32)
    return new_h, new_c
